# revision 1
# baseline (speedup 1.0000x reference)
"""Trainium2 Bass kernel for nn_Chambers (6-tower MLP + coupled sigmoid recurrence).

Data-parallel over 8 NeuronCores: each core processes a 16384-sample shard in
16 chunks of 1024 samples. res tiles are PE-transposed (fp32, exact) into
[100, 512] activation halves; the 4 MLP layers run as fp32r matmuls (full PE
rate at N=512) with chambers packed block-diagonally in L3; L4 accumulates
into a persistent [96, 1024] PSUM tile using per-chunk W4 stacks whose output
column block is 6*chunk (rows outside the block accumulate zeros), which
sidesteps the engines' partition-offset alignment restriction. ACT applies
SiLU+bias straight out of PSUM. The 5-step coupled sigmoid recurrence runs on
the resident [96, 1024] raw tile via a block-diagonal [96,96] matmul.

Sync discipline: this walrus build allows at most 1 sem wait + 1 update per
engine instruction. Constants arrive in single DMAs (one fp32r pack for PE,
one fp32 pack for identity/biases); "touch" ops pre-observe DMA-lane sems;
single-dep nop chains absorb all other cross-engine and same-engine-WAW
waits so no instruction ever needs two.
"""
import numpy as np

import concourse.bass as bass
import concourse.mybir as mybir
from concourse.bass_utils import run_bass_kernel_spmd
from concourse.tile import TileContext
from concourse.tile_scheduler import N_PROCS
from concourse.vector_clock import ScopedClock
from bass_rust import add_dep_helper

F32 = mybir.dt.float32
F32R = mybir.dt.float32r
AF = mybir.ActivationFunctionType
ALU = mybir.AluOpType

# All gpsimd (SWDGE) DMAs share one completion-sem lane so consumers of the
# DMA-assembled raw tile carry a single wait.
import concourse.tile_sem_assignment as _tsa
if not getattr(_tsa.TileClockTick, "_single_swdge", False):
    _orig_tick_init = _tsa.TileClockTick.__init__

    def _tick_init(self, *a, **kw):
        _orig_tick_init(self, *a, **kw)
        self.swdge_sem_count = 1

    _tsa.TileClockTick.__init__ = _tick_init
    _tsa.TileClockTick._single_swdge = True

B = 131072
NCORES = 8
BS = B // NCORES           # 16384 samples per core
T = 1024                   # chunk (samples)
NCH = BS // T              # 16 chunks
RES_DIM = 100
CF_ITERS = 5
CF_K = 0.02

# wf (fp32) column layout
IDC = 0        # identity [128,128]
B1C = 128      # 6 cols
B2C = 134      # 6 cols ([0:64] per chamber)
B3C = 140      # 3 cols ([0:64] per pair)
B4C = 143      # 1 col (b4 tiled x16 over 96 rows)
B2PC = 144     # 3 cols (pair-packed b2: rows 0:64=b2[2pr], 64:128=b2[2pr+1])
B3PC = 147     # 1 col (b3 chambers 0-3 by 32s)
FCOLS = 148

# wr (fp32r) column layout
W1C = 0                    # 6*128
W2C = 768                  # 6*64
W3C = 1152                 # 3*64
CDC = 1344                 # 96
I96C = 1440                # 96 (identity, for raw+delta accumulate)
W4AC = 1536                # 16*96 (per-chunk stacks, chambers 0-3)
W4BC = W4AC + 16 * 96      # 16*96 (per-chunk stacks, chambers 4-5)
W2BC = W4BC + 16 * 96      # 3*128: odd-chamber W2 shifted to out rows 64:127
W3BC = W2BC + 3 * 128      # 128: pair-1 W3 shifted to out rows 64:127
RCOLS = W3BC + 128


class TC(TileContext):
    """TileContext with a walrus-compatible epilogue (split final waits)."""

    def _drain_and_barrier(self, tick_clock, wait_clock):
        nc = self.nc
        full = ScopedClock({None: tick_clock.global_clock})
        for scope, vc in full.items():
            for proc in range(N_PROCS):
                t = vc.peek_next(proc) - 1
                if t > 0:
                    sc = ScopedClock()
                    sc.require_at_least(scope, proc, t)
                    w = nc.sync.nop(nofuse=True)
                    wait_clock.add_sem_waits(w.ins, sc)
        for eng in nc.engines.values():
            eng.drain(fusable=False)
        nc.all_engine_barrier(sem_only=True)
        assert self.sems is not None
        popped = nc._tile_sem_poison_stack.pop()
        assert popped is self._sem_poison
        nc.clear_and_free_semaphores(list(self.sems.allocated().values()))
        for eng in nc.engines.values():
            eng.drain(fusable=False)
        nc.all_engine_barrier(sem_only=True)


def _absorb(eng, deps, after=None):
    """Chain of single-wait nops on `eng`, ordered after `after` if given.
    Returns the last nop (or `after` if no deps)."""
    last = after
    for d in deps:
        if d is None:
            continue
        n = eng.nop(nofuse=True)
        add_dep_helper(n.ins, d.ins, sync=True, reason="absorb")
        if last is not None:
            add_dep_helper(n.ins, last.ins, sync=False, reason="absorb-chain")
        last = n
    return last


def _order(after_inst, before_inst):
    if after_inst is not None and before_inst is not None:
        add_dep_helper(after_inst.ins, before_inst.ins, sync=False, reason="order")


def build_module():
    nc = bass.Bass()
    res_d = nc.dram_tensor("res", [BS, RES_DIM], F32, kind="ExternalInput")
    wf_d = nc.dram_tensor("wf", [128, FCOLS], F32, kind="ExternalInput")
    wr_d = nc.dram_tensor("wr", [128, RCOLS], F32R, kind="ExternalInput")
    raw_d = nc.dram_tensor("raw_out", [96, T], F32, kind="ExternalOutput")
    act_d = nc.dram_tensor("act_out", [96, T], F32, kind="ExternalOutput")

    MMB = 3  # bufs on the shared matmul psum tag

    with TC(nc) as tc:
        with (
            tc.tile_pool(name="wconst", bufs=1) as wpool,
            tc.tile_pool(name="sbres", bufs=1) as sbres,
            tc.tile_pool(name="sbrt", bufs=4) as sbrt,
            tc.tile_pool(name="sbh", bufs=2) as sbh,
            tc.tile_pool(name="sbrec", bufs=1) as sbrec,
            tc.tile_pool(name="pstr", bufs=1, space="PSUM") as pstr,
            tc.tile_pool(name="psscr", bufs=1, space="PSUM") as psscr,
            tc.tile_pool(name="psmm", bufs=MMB, space="PSUM") as psmm,
        ):
            # DMA issue order matters: chunk-0 res and the L1 weights
            # first so compute starts ~4us in; the bulky remainder of the
            # weight pack and later res chunks stream behind.
            res_sb0 = wpool.tile([128, 8 * RES_DIM], F32)
            nc.sync.dma_start(
                out=res_sb0[:],
                in_=res_d[0:T].rearrange("(p n) d -> p (n d)", p=128))
            wf = wpool.tile([128, FCOLS], F32)
            nc.sync.dma_start(out=wf[:], in_=wf_d[:])
            wr = wpool.tile([128, RCOLS], F32R)
            nc.sync.dma_start(out=wr[:, 0:W2C], in_=wr_d[:, 0:W2C])
            res_sb1 = wpool.tile([128, 3 * 8 * RES_DIM], F32)
            nc.sync.dma_start(
                out=res_sb1[:],
                in_=res_d[T:4 * T].rearrange("(p n) d -> p (n d)", p=128))
            nc.sync.dma_start(out=wr[:, W2C:], in_=wr_d[:, W2C:])
            res_sb = wpool.tile([128, (NCH - 4) * 8 * RES_DIM], F32)
            nc.sync.dma_start(
                out=res_sb[:],
                in_=res_d[4 * T:].rearrange("(p n) d -> p (n d)", p=128))
            ident = wf[:, IDC:IDC + 128]

            raw_sb = sbrec.tile([96, T], F32)
            act_r = sbrec.tile([96, T], F32R)
            tmp_sb = sbrec.tile([96, T], F32)
            act_o = sbrec.tile([96, T], F32)
            scr = sbrec.tile([1, 2], F32)
            scrA = sbrec.tile([1, 512], F32)
            scrA2 = sbrec.tile([96, 16], F32)
            scrP = sbrec.tile([1, 16], F32)
            scrD = sbrec.tile([1, 128], F32)

            ps_scr = psscr.tile([128, 512], F32)  # row 0: touch scratch cells

            # PE touch ops: observe the two const DMA lanes (1 wait each)
            warm_r = nc.tensor.matmul(ps_scr[0:1, 496:498], wr[0:1, 0:1],
                                      wr[0:1, 0:2], start=True, stop=True)
            warm_f = nc.tensor.matmul(ps_scr[0:1, 498:500], wf[0:1, 0:1],
                                      wf[0:1, 0:2], start=True, stop=True)
            _order(warm_f, warm_r)
            # ACT touch op: observe the wf DMA lane
            nc.scalar.activation(scr[0:1, 0:1], wf[0:1, B1C:B1C + 1], AF.Copy)

            # Rolling state. Rule: each instruction carries at most one
            # sem wait (its own-engine wait); every cross-engine dependency
            # is pre-observed by a real "touch" instruction (1x2 matmul on
            # PE, 1-elem copy/activation on DVE/ACT) reading the producer's
            # tile. PSUM matmul tiles are [128,1024] (2 banks) on two
            # rotating single-buffer tags: slot reuse is deterministic
            # (k-2) and the pre-touch waits on a silu that has already
            # retired, so ACT streams back-to-back.
            tr_state = []
            pe_tail = warm_f
            act_tail = None
            dve_tail = None
            tcol = [0]
            acol = [0]
            dcol = [0]

            def pe_touch(src_ap):
                nonlocal pe_tail
                t = tcol[0]; tcol[0] += 1
                assert t < 248
                col = 2 * t
                m = nc.tensor.matmul(ps_scr[0:1, col:col + 2],
                                     src_ap[:, 0:1], src_ap[:, 0:2],
                                     start=True, stop=True)
                _order(m, pe_tail)
                pe_tail = m
                return m

            def act_touch(src_ap):
                nonlocal act_tail
                t = acol[0]; acol[0] += 1
                s = nc.scalar.activation(scrA[0:1, t:t + 1], src_ap, AF.Copy)
                _order(s, act_tail)
                act_tail = s
                return s

            def dve_touch(src_ap):
                nonlocal dve_tail
                t = dcol[0]; dcol[0] += 1
                c = nc.vector.tensor_copy(scrD[0:1, t:t + 1], src_ap)
                _order(c, dve_tail)
                dve_tail = c
                return c

            tag_rr = [0]
            tag_state = [None, None, None]

            def new_mm_tile(name, touch=True, width=T):
                nonlocal pe_tail
                tg = tag_rr[0] % 3
                tag_rr[0] += 1
                st = tag_state[tg]
                if st is not None:
                    if touch:
                        tile_, row_, col_ = st
                        pe_touch(tile_[row_:row_ + 1, col_:col_ + 2])
                    tag_state[tg] = None
                t = psmm.tile([128, width], F32, tag=f"mm{tg}", bufs=1,
                              name=name)
                return t, tg

            def mm(out_ap, lhs_ap, rhs_ap, **kw):
                nonlocal pe_tail
                m = nc.tensor.matmul(out_ap, lhs_ap, rhs_ap, **kw)
                _order(m, pe_tail)
                pe_tail = m
                return m

            def set_act_tail(s):
                nonlocal act_tail
                act_tail = s

            def silu(out_ap, pm_ap, bias_ap, out_tile, tg, row, col):
                nonlocal act_tail, act_tile
                s = nc.scalar.activation(out_ap, pm_ap, AF.Silu, bias=bias_ap)
                _order(s, act_tail)
                act_tail = s
                act_tile = out_tile
                tag_state[tg] = (out_tile, row, col)
                return s

            act_tile = None

            def emit_tr(i):
                nonlocal pe_tail, dve_tail
                if i == 0:
                    rq, coff = res_sb0, 0
                elif i < 4:
                    rq, coff = res_sb1, (i - 1) * 8 * RES_DIM
                else:
                    rq, coff = res_sb, (i - 4) * 8 * RES_DIM
                if i in (1, 4):
                    cell = 504 if i == 1 else 508
                    m_ = nc.tensor.matmul(ps_scr[0:1, cell:cell + 2],
                                          rq[0:1, 0:1], rq[0:1, 0:2],
                                          start=True, stop=True)
                    _order(m_, pe_tail)
                    pe_tail = m_
                rTs = []
                for h in range(2):
                    ptr = pstr.tile([100, 512], F32, tag="tr", name="ptr")
                    last_t = None
                    for n in range(4):
                        nn_ = 4 * h + n
                        t_ = nc.tensor.transpose(
                            ptr[:, n * 128:(n + 1) * 128],
                            rq[:, coff + nn_ * RES_DIM:coff + (nn_ + 1) * RES_DIM],
                            ident,
                        )
                        _order(t_, pe_tail)
                        pe_tail = t_
                        last_t = t_
                    rT = sbrt.tile([100, 512], F32R, tag="rT", name="rT")
                    dve_touch(ptr[0:1, 0:1])
                    cp = nc.vector.tensor_copy(rT[:], ptr[:])
                    _order(cp, dve_tail)
                    dve_tail = cp
                    tr_state.append((last_t, cp))
                    rTs.append(rT)
                    pe_touch(rT[0:1, 0:2])
                return rTs

            rts_next = emit_tr(0)
            pending_l4 = []
            for i in range(NCH):
                rTs = rts_next

                # L1: 3 chamber-pairs, one [128,1024] tile per chamber
                h1s = []
                for cp in range(3):
                    ha = sbh.tile([128, T], F32R, tag="h1", bufs=7, name="h1a")
                    hb = sbh.tile([128, T], F32R, tag="h1", bufs=7, name="h1b")
                    pa, ta = new_mm_tile("pm1", touch=False)
                    pb, tb = new_mm_tile("pm1")
                    for s in range(2):
                        mm(pa[:, s * 512:(s + 1) * 512],
                           wr[0:100, W1C + 2 * cp * 128:W1C + (2 * cp + 1) * 128],
                           rTs[s][:], start=True, stop=True)
                    for s in range(2):
                        mm(pb[:, s * 512:(s + 1) * 512],
                           wr[0:100, W1C + (2 * cp + 1) * 128:W1C + (2 * cp + 2) * 128],
                           rTs[s][:], start=True, stop=True)
                    act_touch(pb[0:1, 512:513])
                    silu(ha[:], pa[:], wf[:, B1C + 2 * cp:B1C + 2 * cp + 1],
                         ha, ta, 0, 0)
                    silu(hb[:], pb[:], wf[:, B1C + 2 * cp + 1:B1C + 2 * cp + 2],
                         hb, tb, 0, 0)
                    h1s.extend([ha, hb])



                # L2: per pair, one [64,1024] region per chamber
                if i == 0:
                    # observe the second wr segment's lane just before L2
                    # first needs it (keeps it off the startup critical path)
                    w2 = nc.tensor.matmul(ps_scr[0:1, 492:494],
                                          wr[0:1, W2C:W2C + 1],
                                          wr[0:1, W2C:W2C + 2],
                                          start=True, stop=True)
                    _order(w2, pe_tail)
                    pe_tail = w2
                h2s = []
                l2t = []
                for pr in range(3):
                    pm2, tg2 = new_mm_tile("pm2")
                    for s in range(2):
                        mm(pm2[:, s * 512:(s + 1) * 512],
                           wr[:, W2BC + pr * 128:W2BC + (pr + 1) * 128],
                           h1s[2 * pr + 1][:, s * 512:(s + 1) * 512],
                           start=True, stop=False)
                        mm(pm2[0:64, s * 512:(s + 1) * 512],
                           wr[:, W2C + 2 * pr * 64:W2C + (2 * pr + 1) * 64],
                           h1s[2 * pr][:, s * 512:(s + 1) * 512],
                           start=False, stop=True)
                    l2t.append((pm2, tg2))
                for pr in range(3):
                    pm2, tg2 = l2t[pr]
                    if pr == 0:
                        act_touch(pm2[0:1, 512:513])
                    h2 = sbh.tile([128, T], F32R, tag="h2", bufs=4, name="h2")
                    silu(h2[:], pm2[:], wf[:, B2PC + pr:B2PC + pr + 1],
                         h2, tg2, 0, 0)
                    h2s.append(h2)

                if i + 1 < NCH:
                    rts_next = emit_tr(i + 1)
                if pending_l4:
                    pending_l4.pop(0)()
                # L3: pairs 0,1 merged into one tile; pair 2 separate
                h3a = sbh.tile([128, T], F32R, tag="h3", bufs=4, name="h3a")
                h3b = sbh.tile([128, T], F32R, tag="h3", bufs=4, name="h3b")
                pa, ta = new_mm_tile("pm3", touch=False)
                pc, tc_ = new_mm_tile("pm3b")
                for s in range(2):
                    mm(pa[:, s * 512:(s + 1) * 512],
                       wr[:, W3BC:W3BC + 128],
                       h2s[1][:, s * 512:(s + 1) * 512], start=True, stop=False)
                    mm(pa[0:64, s * 512:(s + 1) * 512],
                       wr[:, W3C:W3C + 64],
                       h2s[0][:, s * 512:(s + 1) * 512], start=False, stop=True)
                pe_touch(h2s[2][0:1, 0:2])  # newest h2 silu
                for s in range(2):
                    mm(pc[0:64, s * 512:(s + 1) * 512],
                       wr[:, W3C + 128:W3C + 192],
                       h2s[2][:, s * 512:(s + 1) * 512], start=True, stop=True)
                act_touch(pc[0:1, 512:513])
                silu(h3a[:], pa[:], wf[:, B3PC:B3PC + 1], h3a, ta, 0, 0)
                silu(h3b[0:64, :], pc[0:64, :], wf[0:64, B3C + 2:B3C + 3],
                     h3b, tc_, 0, 0)

                # L4 deferred past the next chunk's L1 block: per-chunk
                # [6,T] raw rows land in a rotation tile (base 0), are
                # bias-copied to SBUF by ACT, then DMA'd (single SWDGE
                # lane) into raw_sb rows 6i..6i+5.
                def emit_l4(i=i, h3a=h3a, h3b=h3b):
                    pe_touch(h3b[0:1, 0:2])   # h3 silus retired by now
                    pm4, tg4 = new_mm_tile("pm4")
                    for s in range(2):
                        mm(pm4[0:6, s * 512:(s + 1) * 512],
                           wr[:, W4AC:W4AC + 6],
                           h3a[:, s * 512:(s + 1) * 512],
                           start=True, stop=False)
                        mm(pm4[0:6, s * 512:(s + 1) * 512],
                           wr[0:64, W4BC:W4BC + 6],
                           h3b[0:64, s * 512:(s + 1) * 512],
                           start=False, stop=True)
                    act_touch(pm4[0:1, 512:513])
                    raw_i = sbh.tile([6, T], F32, tag="rawi", bufs=2,
                                     name="raw_i")
                    ro = nc.scalar.activation(raw_i[:], pm4[0:6, :],
                                              AF.Identity,
                                              bias=wf[0:6, B4C:B4C + 1])
                    _order(ro, act_tail)
                    set_act_tail(ro)
                    tag_state[tg4] = (raw_i, 0, 0)
                    # ACT observes the assembly DMAs (covers the raw_i slot
                    # WAR two chunks later); Pool observes ACT through it
                    s_ = nc.scalar.activation(scrA2[:, (i % 16):(i % 16) + 1],
                                              raw_sb[0:96, 0:1], AF.Copy)
                    _order(s_, act_tail)
                    set_act_tail(s_)
                    nc.gpsimd.tensor_copy(scrP[0:1, (i % 16):(i % 16) + 1],
                                          scrA2[0:1, (i % 16):(i % 16) + 1])
                    nc.gpsimd.dma_start(out=raw_sb[6 * i:6 * i + 6, :],
                                        in_=raw_i[:])
                pending_l4.append(emit_l4)

            if pending_l4:
                pending_l4.pop(0)()

            # ---- coupled sigmoid recurrence on [96, T] ----
            raw_r = sbrec.tile([96, T], F32R)
            cpr = nc.vector.tensor_copy(raw_r[:], raw_sb[:])
            _order(cpr, dve_tail)
            dve_tail = cpr
            pe_touch(raw_r[0:1, 0:2])
            sig = nc.scalar.activation(act_r[:], raw_sb[:], AF.Sigmoid)
            _order(sig, act_tail)
            act_tail = sig
            for kk in range(CF_ITERS):
                dst = act_r if kk < CF_ITERS - 1 else act_o
                pe_touch(act_r[0:1, 0:2])   # PE observes the latest sigmoid
                for s in range(2):
                    pm5, tg5 = new_mm_tile("pm5", touch=False, width=512)
                    mm(pm5[0:96, 0:512],
                       wr[0:96, CDC:CDC + 96],
                       act_r[:, s * 512:(s + 1) * 512],
                       start=True, stop=False)
                    mm(pm5[0:96, 0:512],
                       wr[0:96, I96C:I96C + 96],
                       raw_r[:, s * 512:(s + 1) * 512],
                       start=False, stop=True)
                    act_touch(pm5[0:1, 0:1])
                    sig = nc.scalar.activation(
                        dst[:, s * 512:(s + 1) * 512], pm5[0:96, 0:512],
                        AF.Sigmoid)
                    _order(sig, act_tail)
                    act_tail = sig
                    tag_state[tg5] = (dst, 0, s * 512)

            nc.sync.dma_start(out=raw_d[:], in_=raw_sb[:])
            nc.sync.dma_start(out=act_d[:], in_=act_o[:])

    return nc


def _pack_consts(W1, b1, W2, b2, W3, b3, W4, b4, coupling, decay):
    wf = np.zeros((128, FCOLS), dtype=np.float32)
    wf[:, IDC:IDC + 128] = np.eye(128, dtype=np.float32)
    for c in range(6):
        wf[:, B1C + c] = b1[c]
    for c in range(6):
        wf[0:64, B2C + c] = b2[c]
    for pr in range(3):
        wf[0:32, B3C + pr] = b3[2 * pr]
        wf[32:64, B3C + pr] = b3[2 * pr + 1]
    wf[0:96, B4C] = np.tile(b4, 16)

    wr = np.zeros((128, RCOLS), dtype=np.float32)
    for c in range(6):
        wr[0:100, W1C + c * 128:W1C + (c + 1) * 128] = W1[c]
        wr[0:128, W2C + c * 64:W2C + (c + 1) * 64] = W2[c]
    for pr in range(3):
        wr[0:64, W3C + pr * 64:W3C + pr * 64 + 32] = W3[2 * pr]
        wr[64:128, W3C + pr * 64 + 32:W3C + (pr + 1) * 64] = W3[2 * pr + 1]
    cd = (decay[:, None] * coupling * CF_K).astype(np.float32)
    for g in range(16):
        wr[6 * g:6 * g + 6, CDC + 6 * g:CDC + 6 * g + 6] = cd
    wr[0:96, I96C:I96C + 96] = np.eye(96, dtype=np.float32)
    for c in range(4):
        wr[c * 32:(c + 1) * 32, W4AC + c] = W4[c]
    for c2 in range(2):
        wr[c2 * 32:(c2 + 1) * 32, W4BC + 4 + c2] = W4[4 + c2]
    # odd chambers of each L2 pair, shifted to output rows 64:127 (cols
    # 0:64 stay zero so start=True clears the even chamber's rows for the
    # accumulating second matmul)
    for pr in range(3):
        wr[0:128, W2BC + pr * 128 + 64:W2BC + (pr + 1) * 128] = W2[2 * pr + 1]
        wf[0:64, B2PC + pr] = b2[2 * pr]
        wf[64:128, B2PC + pr] = b2[2 * pr + 1]
    # L3 pair 1 (chambers 2,3) shifted to rows 64:127 of the merged tile
    wr[0:64, W3BC + 64:W3BC + 96] = W3[2]
    wr[64:128, W3BC + 96:W3BC + 128] = W3[3]
    for c in range(4):
        wf[c * 32:(c + 1) * 32, B3PC] = b3[c]
    return wf, wr


def _unshard(per_core, key):
    """[96, T] group layout -> [BS, 6] per core, concat to [B, 6].

    Chunk 0: sample p*8+n8. Chunks 1-3: 1024 + p*24 + (i-1)*8 + n8.
    Chunks 4-15: 4096 + p*96 + (i-4)*8 + n8."""
    outs = []
    for r in per_core:
        a = r[key].reshape(NCH, 6, 8, 128)             # [i, c, n8, p]
        out = np.empty((BS, 6), dtype=a.dtype)
        out[0:T] = a[0].transpose(2, 1, 0).reshape(T, 6)
        out[T:4 * T] = a[1:4].transpose(3, 0, 2, 1).reshape(3 * T, 6)
        out[4 * T:] = a[4:].transpose(3, 0, 2, 1).reshape(12 * T, 6)
        outs.append(out)
    return np.concatenate(outs, axis=0)


def kernel(res, W1, b1, W2, b2, W3, b3, W4, b4, coupling, decay):
    res = np.asarray(res, dtype=np.float32)
    args = [np.asarray(a, dtype=np.float32)
            for a in (W1, b1, W2, b2, W3, b3, W4, b4, coupling, decay)]
    wf, wr = _pack_consts(*args)

    nc = build_module()
    in_maps = [
        {"res": np.ascontiguousarray(res[i * BS:(i + 1) * BS]), "wf": wf, "wr": wr}
        for i in range(NCORES)
    ]
    results = run_bass_kernel_spmd(nc, in_maps, core_ids=list(range(NCORES)))
    act = _unshard(results.results, "act_out")
    raw = _unshard(results.results, "raw_out")
    return act, raw



# revision 2
# speedup vs baseline: 1.0241x; 1.0241x over previous
"""Trainium2 Bass kernel for nn_Chambers (6-tower MLP + coupled sigmoid recurrence).

Data-parallel over 8 NeuronCores: each core processes a 16384-sample shard in
16 chunks of 1024 samples. v2 design:

- bf16 matmul pipeline: res is converted fp32->bf16 on DVE, PE-transposed in
  bf16 (1.0 cycles/row vs 2.0 for fp32), and L1-L4 run as bf16 matmuls with
  chambers packed block-diagonally. All h-activations are bf16 in SBUF.
- L3 chamber pair (4,5) is sample-folded into a [128, 512] tile (samples
  0:511 on partitions 0:63, 512:1023 on 64:127) so its SiLU costs 512 ACT
  rows instead of 1024.
- L4 accumulates raw directly into a persistent [96, 1024] PSUM tile across
  all 16 chunks via per-chunk W4 column stacks (chunk i writes rows 6i:6i+5;
  other rows accumulate zeros). This removes the per-chunk raw bias-copy on
  ACT and all SWDGE assembly DMAs.
- The 5-step coupled sigmoid recurrence runs on the resident raw tile via a
  block-diagonal [96,96] f32r matmul, as in v1.

Sync discipline (walrus: at most 1 sem wait + 1 update per instruction):
PSUM tag rotation is pre-observed by 1x2 "touch" matmuls on PE; all other
cross-engine deps resolve to a single auto-added wait because each
instruction's data wait subsumes its WAR wait on the same engine-sem lane.
"""
import numpy as np
import ml_dtypes

import concourse.bass as bass
import concourse.mybir as mybir
from concourse.bass_utils import run_bass_kernel_spmd
from concourse.tile import TileContext
from concourse.tile_scheduler import N_PROCS
from concourse.vector_clock import ScopedClock
from bass_rust import add_dep_helper

F32 = mybir.dt.float32
F32R = mybir.dt.float32r
BF16 = mybir.dt.bfloat16
AF = mybir.ActivationFunctionType
ALU = mybir.AluOpType

B = 131072
NCORES = 8
BS = B // NCORES           # 16384 samples per core
T = 1024                   # chunk (samples)
NCH = BS // T              # 16 chunks
RES_DIM = 100
CF_ITERS = 5
CF_K = 0.02

# wf (fp32) column layout: per-partition bias vectors
B1C = 0        # 6 cols (b1 per chamber, 128 rows)
B2PC = 6       # 3 cols (pair-packed b2: rows 0:64=b2[2pr], 64:128=b2[2pr+1])
B3PC = 9       # 1 col (b3 chambers 0-3 by 32s)
B3P2C = 10     # 1 col (b3 ch4,5 folded twice over 128 rows)
B4C = 11       # 1 col (b4 tiled x16 over 96 rows)
FCOLS = 12

# wr (fp32r): recurrence matrices
CDC = 0        # 96 (block-diag decay*coupling*k)
I96C = 96      # 96 (identity, for raw+delta accumulate)
RCOLS = 192

# wb (bf16): matmul weights
IDC = 0                    # identity [128,128] for PE transpose
W1C = 128                  # 6*128
W2C = W1C + 6 * 128        # 6*64
W2BC = W2C + 6 * 64        # 3*128: odd-chamber W2 shifted to out rows 64:127
W3AC = W2BC + 3 * 128      # 64: pair 0 (both chambers packed on K)
W3BC = W3AC + 64           # 128: pair 1 shifted to out rows 64:127
W3CC = W3BC + 128          # 64: pair 2
W4AC = W3CC + 64           # 16*96 per-chunk stacks (chambers 0-3)
W4BC = W4AC + 16 * 96      # 16*96 per-chunk stacks (ch 4,5; both sample halves)
BCOLS = W4BC + 16 * 96
WB_SPLIT = W4AC            # early cols: ident..W3; late: W4 stacks


class TC(TileContext):
    """TileContext with a walrus-compatible epilogue (split final waits)."""

    def _drain_and_barrier(self, tick_clock, wait_clock):
        nc = self.nc
        full = ScopedClock({None: tick_clock.global_clock})
        for scope, vc in full.items():
            for proc in range(N_PROCS):
                t = vc.peek_next(proc) - 1
                if t > 0:
                    sc = ScopedClock()
                    sc.require_at_least(scope, proc, t)
                    w = nc.sync.nop(nofuse=True)
                    wait_clock.add_sem_waits(w.ins, sc)
        for eng in nc.engines.values():
            eng.drain(fusable=False)
        nc.all_engine_barrier(sem_only=True)
        assert self.sems is not None
        popped = nc._tile_sem_poison_stack.pop()
        assert popped is self._sem_poison
        nc.clear_and_free_semaphores(list(self.sems.allocated().values()))
        for eng in nc.engines.values():
            eng.drain(fusable=False)
        nc.all_engine_barrier(sem_only=True)


def _order(after_inst, before_inst):
    if after_inst is not None and before_inst is not None:
        add_dep_helper(after_inst.ins, before_inst.ins, sync=False, reason="order")


def build_module():
    nc = bass.Bass()
    res_d = nc.dram_tensor("res", [BS, RES_DIM], F32, kind="ExternalInput")
    wf_d = nc.dram_tensor("wf", [128, FCOLS], F32, kind="ExternalInput")
    wr_d = nc.dram_tensor("wr", [128, RCOLS], F32R, kind="ExternalInput")
    wb_d = nc.dram_tensor("wb", [128, BCOLS], BF16, kind="ExternalInput")
    raw_d = nc.dram_tensor("raw_out", [96, T], F32, kind="ExternalOutput")
    act_d = nc.dram_tensor("act_out", [96, T], F32, kind="ExternalOutput")

    MMB = 2  # rotating matmul psum tags

    with TC(nc) as tc:
        with (
            tc.tile_pool(name="wconst", bufs=1) as wpool,
            tc.tile_pool(name="sbresb", bufs=1) as sbresb,
            tc.tile_pool(name="sbrt", bufs=1) as sbrt,
            tc.tile_pool(name="sbh", bufs=1) as sbh,
            tc.tile_pool(name="sbrec", bufs=1) as sbrec,
            tc.tile_pool(name="pstr", bufs=1, space="PSUM") as pstr,
            tc.tile_pool(name="psscr", bufs=1, space="PSUM") as psscr,
            tc.tile_pool(name="psmm", bufs=MMB, space="PSUM") as psmm,
            tc.tile_pool(name="psl4", bufs=1, space="PSUM") as psl4,
        ):
            # DMA issue order: chunk-0 res + early weights first so compute
            # starts quickly; bulky W4 stacks and later res chunks stream
            # behind.
            res_sb0 = wpool.tile([128, 8 * RES_DIM], F32)
            nc.sync.dma_start(
                out=res_sb0[:],
                in_=res_d[0:T].rearrange("(p n) d -> p (n d)", p=128))
            wf = wpool.tile([128, FCOLS], F32)
            nc.sync.dma_start(out=wf[:], in_=wf_d[:])
            wb = wpool.tile([128, BCOLS], BF16)
            nc.sync.dma_start(out=wb[:, 0:WB_SPLIT], in_=wb_d[:, 0:WB_SPLIT])
            res_sb1 = wpool.tile([128, 3 * 8 * RES_DIM], F32)
            nc.sync.dma_start(
                out=res_sb1[:],
                in_=res_d[T:4 * T].rearrange("(p n) d -> p (n d)", p=128))
            nc.sync.dma_start(out=wb[:, WB_SPLIT:], in_=wb_d[:, WB_SPLIT:])
            wr = wpool.tile([128, RCOLS], F32R)
            nc.sync.dma_start(out=wr[:], in_=wr_d[:])
            res_sb = wpool.tile([128, (NCH - 4) * 8 * RES_DIM], F32)
            nc.sync.dma_start(
                out=res_sb[:],
                in_=res_d[4 * T:].rearrange("(p n) d -> p (n d)", p=128))
            ident = wb[:, IDC:IDC + 128]

            raw_sb = sbrec.tile([96, T], F32)
            act_r = sbrec.tile([96, T], F32R)
            act_o = sbrec.tile([96, T], F32)
            raw_r = sbrec.tile([96, T], F32R)
            scr = sbrec.tile([1, 4], F32)
            scrD = sbrec.tile([1, 8], F32)

            ps_scr = psscr.tile([128, 512], F32)   # touch scratch cells
            pm4 = psl4.tile([96, T], F32)          # persistent raw accumulator

            # Startup observes: PE on wb lane, ACT on wf lane, DVE on wf lane.
            warm_b = nc.tensor.matmul(ps_scr[0:1, 498:500], wb[0:1, 0:1],
                                      wb[0:1, 0:2], start=True, stop=True)
            nc.scalar.activation(scr[0:1, 0:1], wf[0:1, B1C:B1C + 1], AF.Copy)
            nc.vector.tensor_copy(scrD[0:1, 0:1], wf[0:1, B1C:B1C + 1])

            pe_tail = warm_b
            act_tail = None
            dve_tail = None
            tcol = [0]

            def pe_touch(src_ap):
                nonlocal pe_tail
                t = tcol[0]; tcol[0] += 1
                assert t < 248
                col = 2 * t
                m = nc.tensor.matmul(ps_scr[0:1, col:col + 2],
                                     src_ap[:, 0:1], src_ap[:, 0:2],
                                     start=True, stop=True)
                _order(m, pe_tail)
                pe_tail = m
                return m

            tag_rr = [0]
            tag_state = [None] * MMB

            def new_mm_tile(name, touch=True, width=T):
                tg = tag_rr[0] % MMB
                tag_rr[0] += 1
                st = tag_state[tg]
                if st is not None:
                    if touch:
                        tile_, row_, col_ = st
                        pe_touch(tile_[row_:row_ + 1, col_:col_ + 2])
                    tag_state[tg] = None
                t = psmm.tile([128, width], F32, tag=f"mm{tg}", bufs=1,
                              name=name)
                return t, tg

            def mm(out_ap, lhs_ap, rhs_ap, **kw):
                nonlocal pe_tail
                m = nc.tensor.matmul(out_ap, lhs_ap, rhs_ap, **kw)
                _order(m, pe_tail)
                pe_tail = m
                return m

            def silu(out_ap, pm_ap, bias_ap, out_tile, tg, row, col,
                     func=AF.Silu):
                nonlocal act_tail
                s = nc.scalar.activation(out_ap, pm_ap, func, bias=bias_ap)
                _order(s, act_tail)
                act_tail = s
                if tg is not None:
                    tag_state[tg] = (out_tile, row, col)
                return s

            def dve(op):
                nonlocal dve_tail
                _order(op, dve_tail)
                dve_tail = op
                return op

            def res_src(i):
                if i == 0:
                    return res_sb0, 0
                if i < 4:
                    return res_sb1, (i - 1) * 8 * RES_DIM
                return res_sb, (i - 4) * 8 * RES_DIM

            def emit_conv(i):
                """DVE: convert chunk i's res slice fp32 -> bf16."""
                rq, coff = res_src(i)
                rb = sbresb.tile([128, 8 * RES_DIM], BF16, tag="rb", bufs=2,
                                 name="rb")
                cp = dve(nc.vector.tensor_copy(
                    rb[:], rq[:, coff:coff + 8 * RES_DIM]))
                return rb

            def emit_tr(i, rb):
                """PE transposes + DVE copy -> rT [100, 1024] bf16."""
                nonlocal pe_tail
                ptr = pstr.tile([RES_DIM, T], BF16, tag="tr", name="ptr")
                for nn_ in range(8):
                    t_ = nc.tensor.transpose(
                        ptr[:, nn_ * 128:(nn_ + 1) * 128],
                        rb[:, nn_ * RES_DIM:(nn_ + 1) * RES_DIM],
                        ident,
                    )
                    _order(t_, pe_tail)
                    pe_tail = t_
                rT = sbrt.tile([RES_DIM, T], BF16, tag="rT", bufs=3, name="rT")
                cp = dve(nc.vector.tensor_copy(rT[:], ptr[:]))
                return rT

            rb_next = emit_conv(0)
            rt_next = emit_tr(0, rb_next)
            pending_l4 = []
            for i in range(NCH):
                rT = rt_next

                # L1: 6 chambers, one [128,1024] psum tile each
                h1s = []
                for cp3 in range(3):
                    ha = sbh.tile([128, T], BF16, tag="h1", bufs=6, name="h1a")
                    hb = sbh.tile([128, T], BF16, tag="h1", bufs=6, name="h1b")
                    pa, ta = new_mm_tile("pm1a")
                    pb, tb = new_mm_tile("pm1b")
                    ca, cb = 2 * cp3, 2 * cp3 + 1
                    for s in range(2):
                        mm(pa[:, s * 512:(s + 1) * 512],
                           wb[0:RES_DIM, W1C + ca * 128:W1C + (ca + 1) * 128],
                           rT[:, s * 512:(s + 1) * 512], start=True, stop=True)
                    for s in range(2):
                        mm(pb[:, s * 512:(s + 1) * 512],
                           wb[0:RES_DIM, W1C + cb * 128:W1C + (cb + 1) * 128],
                           rT[:, s * 512:(s + 1) * 512], start=True, stop=True)
                    silu(ha[:], pa[:], wf[:, B1C + ca:B1C + ca + 1],
                         ha, ta, 0, 0)
                    silu(hb[:], pb[:], wf[:, B1C + cb:B1C + cb + 1],
                         hb, tb, 0, 0)
                    h1s.extend([ha, hb])

                # next chunk's res conversion can start as soon as DVE is free
                if i + 1 < NCH:
                    rb_next = emit_conv(i + 1)

                # L2: 3 pairs, both chambers stacked on out partitions
                l2t = []
                for pr in range(3):
                    pm2, tg2 = new_mm_tile("pm2")
                    for s in range(2):
                        mm(pm2[:, s * 512:(s + 1) * 512],
                           wb[:, W2BC + pr * 128:W2BC + (pr + 1) * 128],
                           h1s[2 * pr + 1][:, s * 512:(s + 1) * 512],
                           start=True, stop=False)
                        mm(pm2[0:64, s * 512:(s + 1) * 512],
                           wb[:, W2C + 2 * pr * 64:W2C + (2 * pr + 1) * 64],
                           h1s[2 * pr][:, s * 512:(s + 1) * 512],
                           start=False, stop=True)
                    l2t.append((pm2, tg2))
                h2s = []
                for pr in range(3):
                    pm2, tg2 = l2t[pr]
                    h2 = sbh.tile([128, T], BF16, tag="h2", bufs=4, name="h2")
                    silu(h2[:], pm2[:], wf[:, B2PC + pr:B2PC + pr + 1],
                         h2, tg2, 0, 0)
                    h2s.append(h2)

                if i + 1 < NCH:
                    rt_next = emit_tr(i + 1, rb_next)
                if pending_l4:
                    pending_l4.pop(0)()

                # L3: pairs 0,1 merged into one [128,1024] tile; pair 2
                # sample-folded into [128,512]
                h3a = sbh.tile([128, T], BF16, tag="h3a", bufs=3, name="h3a")
                h3b = sbh.tile([128, 512], BF16, tag="h3b", bufs=3, name="h3b")
                pa3, ta3 = new_mm_tile("pm3")
                pc3, tc3 = new_mm_tile("pm3b", width=512)
                for s in range(2):
                    mm(pa3[:, s * 512:(s + 1) * 512],
                       wb[:, W3BC:W3BC + 128],
                       h2s[1][:, s * 512:(s + 1) * 512], start=True, stop=False)
                    mm(pa3[0:64, s * 512:(s + 1) * 512],
                       wb[:, W3AC:W3AC + 64],
                       h2s[0][:, s * 512:(s + 1) * 512], start=False, stop=True)
                mm(pc3[0:64, 0:512], wb[:, W3CC:W3CC + 64],
                   h2s[2][:, 0:512], start=True, stop=False)
                mm(pc3[64:128, 0:512], wb[:, W3CC:W3CC + 64],
                   h2s[2][:, 512:1024], start=False, stop=True)
                silu(h3a[:], pa3[:], wf[:, B3PC:B3PC + 1], h3a, ta3, 0, 0)
                silu(h3b[:], pc3[:], wf[:, B3P2C:B3P2C + 1], h3b, tc3, 0, 0)

                # L4 deferred past the next chunk's L1/L2: accumulate raw
                # rows 6i:6i+6 into the persistent [96,1024] psum tile.
                def emit_l4(i=i, h3a=h3a, h3b=h3b):
                    if i == 0:
                        pe_touch(wb[0:1, W4AC:W4AC + 2])  # observe W4 DMA
                    for s in range(2):
                        mm(pm4[0:96, s * 512:(s + 1) * 512],
                           wb[:, W4AC + 96 * i:W4AC + 96 * (i + 1)],
                           h3a[:, s * 512:(s + 1) * 512],
                           start=(i == 0), stop=False)
                        mm(pm4[0:96, s * 512:(s + 1) * 512],
                           wb[64 * s:64 * s + 64,
                              W4BC + 96 * i:W4BC + 96 * (i + 1)],
                           h3b[64 * s:64 * s + 64, 0:512],
                           start=False, stop=(i == NCH - 1))
                    pe_tail_local = None
                pending_l4.append(emit_l4)

            if pending_l4:
                pending_l4.pop(0)()

            # ---- tail: raw materialization + coupled sigmoid recurrence ----
            cpr = dve(nc.vector.tensor_scalar(
                out=raw_sb[:], in0=pm4[0:96, :],
                scalar1=wf[0:96, B4C:B4C + 1], scalar2=None, op0=ALU.add))
            cprr = dve(nc.vector.tensor_copy(raw_r[:], raw_sb[:]))
            nc.sync.dma_start(out=raw_d[:], in_=raw_sb[:])
            pe_touch(wr[0:1, 0:2])      # observe wr DMA lane
            pe_touch(raw_r[0:1, 0:2])   # observe DVE raw_r
            sig = silu(act_r[:], pm4[0:96, :], wf[0:96, B4C:B4C + 1],
                       None, None, 0, 0, func=AF.Sigmoid)
            for kk in range(CF_ITERS):
                dst = act_r if kk < CF_ITERS - 1 else act_o
                pe_touch(act_r[0:1, 0:2])   # PE observes the latest sigmoid
                for s in range(2):
                    pm5, tg5 = new_mm_tile("pm5", width=512)
                    mm(pm5[0:96, 0:512],
                       wr[0:96, CDC:CDC + 96],
                       act_r[:, s * 512:(s + 1) * 512],
                       start=True, stop=False)
                    mm(pm5[0:96, 0:512],
                       wr[0:96, I96C:I96C + 96],
                       raw_r[:, s * 512:(s + 1) * 512],
                       start=False, stop=True)
                    sg = silu(dst[:, s * 512:(s + 1) * 512], pm5[0:96, 0:512],
                              0.0, dst, tg5, 0, s * 512, func=AF.Sigmoid)
            nc.sync.dma_start(out=act_d[:], in_=act_o[:])

    return nc


def _pack_consts(W1, b1, W2, b2, W3, b3, W4, b4, coupling, decay):
    wf = np.zeros((128, FCOLS), dtype=np.float32)
    for c in range(6):
        wf[:, B1C + c] = b1[c]
    for pr in range(3):
        wf[0:64, B2PC + pr] = b2[2 * pr]
        wf[64:128, B2PC + pr] = b2[2 * pr + 1]
    for c in range(4):
        wf[c * 32:(c + 1) * 32, B3PC] = b3[c]
    for s in range(2):
        wf[64 * s:64 * s + 32, B3P2C] = b3[4]
        wf[64 * s + 32:64 * s + 64, B3P2C] = b3[5]
    wf[0:96, B4C] = np.tile(b4, 16)

    wr = np.zeros((128, RCOLS), dtype=np.float32)
    cd = (decay[:, None] * coupling * CF_K).astype(np.float32)
    for g in range(16):
        wr[6 * g:6 * g + 6, CDC + 6 * g:CDC + 6 * g + 6] = cd
    wr[0:96, I96C:I96C + 96] = np.eye(96, dtype=np.float32)

    wb = np.zeros((128, BCOLS), dtype=np.float32)
    wb[:, IDC:IDC + 128] = np.eye(128, dtype=np.float32)
    for c in range(6):
        wb[0:RES_DIM, W1C + c * 128:W1C + (c + 1) * 128] = W1[c]
        wb[0:128, W2C + c * 64:W2C + (c + 1) * 64] = W2[c]
    for pr in range(3):
        # odd chamber shifted to out rows 64:127; cols 0:64 stay zero so
        # start=True clears the even chamber's rows for the accumulate
        wb[:, W2BC + pr * 128 + 64:W2BC + (pr + 1) * 128] = W2[2 * pr + 1]
    wb[0:64, W3AC:W3AC + 32] = W3[0]
    wb[64:128, W3AC + 32:W3AC + 64] = W3[1]
    wb[0:64, W3BC + 64:W3BC + 96] = W3[2]
    wb[64:128, W3BC + 96:W3BC + 128] = W3[3]
    wb[0:64, W3CC:W3CC + 32] = W3[4]
    wb[64:128, W3CC + 32:W3CC + 64] = W3[5]
    for i in range(16):
        ba = W4AC + 96 * i
        for c in range(4):
            wb[c * 32:(c + 1) * 32, ba + 6 * i + c] = W4[c]
        bb = W4BC + 96 * i
        for s in range(2):
            wb[64 * s:64 * s + 32, bb + 6 * i + 4] = W4[4]
            wb[64 * s + 32:64 * s + 64, bb + 6 * i + 5] = W4[5]
    return wf, wr, wb.astype(ml_dtypes.bfloat16)


def _unshard(per_core, key):
    """[96, T] group layout -> [BS, 6] per core, concat to [B, 6].

    Chunk 0: sample p*8+n8. Chunks 1-3: 1024 + p*24 + (i-1)*8 + n8.
    Chunks 4-15: 4096 + p*96 + (i-4)*8 + n8."""
    outs = []
    for r in per_core:
        a = r[key].reshape(NCH, 6, 8, 128)             # [i, c, n8, p]
        out = np.empty((BS, 6), dtype=a.dtype)
        out[0:T] = a[0].transpose(2, 1, 0).reshape(T, 6)
        out[T:4 * T] = a[1:4].transpose(3, 0, 2, 1).reshape(3 * T, 6)
        out[4 * T:] = a[4:].transpose(3, 0, 2, 1).reshape(12 * T, 6)
        outs.append(out)
    return np.concatenate(outs, axis=0)


def kernel(res, W1, b1, W2, b2, W3, b3, W4, b4, coupling, decay):
    res = np.asarray(res, dtype=np.float32)
    args = [np.asarray(a, dtype=np.float32)
            for a in (W1, b1, W2, b2, W3, b3, W4, b4, coupling, decay)]
    wf, wr, wb = _pack_consts(*args)

    nc = build_module()
    in_maps = [
        {"res": np.ascontiguousarray(res[i * BS:(i + 1) * BS]),
         "wf": wf, "wr": wr, "wb": wb}
        for i in range(NCORES)
    ]
    results = run_bass_kernel_spmd(nc, in_maps, core_ids=list(range(NCORES)))
    act = _unshard(results.results, "act_out")
    raw = _unshard(results.results, "raw_out")
    return act, raw


# revision 3
# speedup vs baseline: 1.1672x; 1.1398x over previous
"""Trainium2 Bass kernel for nn_Chambers (6-tower MLP + coupled sigmoid recurrence).

Data-parallel over 8 NeuronCores: each core processes a 16384-sample shard in
16 chunks of 1024 samples. v2 design:

- bf16 matmul pipeline: res is converted fp32->bf16 on DVE, PE-transposed in
  bf16 (1.0 cycles/row vs 2.0 for fp32), and L1-L4 run as bf16 matmuls with
  chambers packed block-diagonally. All h-activations are bf16 in SBUF.
- L3 chamber pair (4,5) is sample-folded into a [128, 512] tile (samples
  0:511 on partitions 0:63, 512:1023 on 64:127) so its SiLU costs 512 ACT
  rows instead of 1024.
- L4 accumulates raw directly into a persistent [96, 1024] PSUM tile across
  all 16 chunks via per-chunk W4 column stacks (chunk i writes rows 6i:6i+5;
  other rows accumulate zeros). This removes the per-chunk raw bias-copy on
  ACT and all SWDGE assembly DMAs.
- PSUM: 3 rotating [128,1024] matmul tags (6 banks) + the persistent raw
  accumulator (2 banks). Transpose tiles ride the same tag rotation (bf16,
  half a slot); PE "touch" matmuls write into the tag tile being allocated,
  so no scratch bank is needed.
- The 5-step coupled sigmoid recurrence runs on the resident raw tile via a
  block-diagonal [96,96] f32r matmul.

Sync discipline (walrus: at most 1 sem wait + 1 update per instruction):
PSUM tag rotation is pre-observed by 1x2 "touch" matmuls on PE; all other
cross-engine deps resolve to a single auto-added wait because each
instruction's data wait subsumes its WAR wait on the same engine-sem lane.
"""
import numpy as np
import ml_dtypes

import concourse.bass as bass
import concourse.mybir as mybir
from concourse.bass_utils import run_bass_kernel_spmd
from concourse.tile import TileContext
from concourse.tile_scheduler import N_PROCS
from concourse.vector_clock import ScopedClock
from bass_rust import add_dep_helper

F32 = mybir.dt.float32
F32R = mybir.dt.float32r
BF16 = mybir.dt.bfloat16
AF = mybir.ActivationFunctionType
ALU = mybir.AluOpType

B = 131072
NCORES = 8
BS = B // NCORES           # 16384 samples per core
T = 1024                   # chunk (samples)
NCH = BS // T              # 16 chunks
RES_DIM = 100
CF_ITERS = 5
CF_K = 0.02

# wf (fp32) column layout: per-partition bias vectors
B1C = 0        # 6 cols (b1 per chamber, 128 rows)
B2PC = 6       # 3 cols (pair-packed b2: rows 0:64=b2[2pr], 64:128=b2[2pr+1])
B3PC = 9       # 1 col (b3 chambers 0-3 by 32s)
B3P2C = 10     # 1 col (b3 ch4,5 folded twice over 128 rows)
B4C = 11       # 1 col (b4 tiled x16 over 96 rows)
FCOLS = 12

# wr (fp32r): recurrence matrices
CDC = 0        # 96 (block-diag decay*coupling*k)
I96C = 96      # 96 (identity, for raw+delta accumulate)
RCOLS = 192

# wa (bf16): early matmul weights
IDC = 0                    # identity [128,128] for PE transpose
W1C = 128                  # 6*128
W2C = W1C + 6 * 128        # 6*64
W2BC = W2C + 6 * 64        # 3*128: odd-chamber W2 shifted to out rows 64:127
W3AC = W2BC + 3 * 128      # 64: pair 0 (both chambers packed on K)
W3BC = W3AC + 64           # 128: pair 1 shifted to out rows 64:127
W3CC = W3BC + 128          # 64: pair 2
ACOLS = W3CC + 64

# wz (bf16): per-chunk W4 stacks
W4AC = 0                   # 16*96 (chambers 0-3)
W4BC = 16 * 96             # 16*96 (ch 4,5; both sample halves)
ZCOLS = 2 * 16 * 96


class TC(TileContext):
    """TileContext with a walrus-compatible epilogue (split final waits)."""

    def _drain_and_barrier(self, tick_clock, wait_clock):
        nc = self.nc
        full = ScopedClock({None: tick_clock.global_clock})
        for scope, vc in full.items():
            for proc in range(N_PROCS):
                t = vc.peek_next(proc) - 1
                if t > 0:
                    sc = ScopedClock()
                    sc.require_at_least(scope, proc, t)
                    w = nc.sync.nop(nofuse=True)
                    wait_clock.add_sem_waits(w.ins, sc)
        for eng in nc.engines.values():
            eng.drain(fusable=False)
        nc.all_engine_barrier(sem_only=True)
        assert self.sems is not None
        popped = nc._tile_sem_poison_stack.pop()
        assert popped is self._sem_poison
        nc.clear_and_free_semaphores(list(self.sems.allocated().values()))
        for eng in nc.engines.values():
            eng.drain(fusable=False)
        nc.all_engine_barrier(sem_only=True)


def _order(after_inst, before_inst):
    if after_inst is not None and before_inst is not None:
        add_dep_helper(after_inst.ins, before_inst.ins, sync=False, reason="order")


def build_module():
    nc = bass.Bass()
    res_d = nc.dram_tensor("res", [BS, RES_DIM], F32, kind="ExternalInput")
    wf_d = nc.dram_tensor("wf", [128, FCOLS], F32, kind="ExternalInput")
    wr_d = nc.dram_tensor("wr", [128, RCOLS], F32R, kind="ExternalInput")
    wa_d = nc.dram_tensor("wa", [128, ACOLS], BF16, kind="ExternalInput")
    wz_d = nc.dram_tensor("wz", [128, ZCOLS], BF16, kind="ExternalInput")
    raw_d = nc.dram_tensor("raw_out", [96, T], F32, kind="ExternalOutput")
    act_d = nc.dram_tensor("act_out", [96, T], F32, kind="ExternalOutput")

    MMB = 3  # rotating matmul psum tags

    with TC(nc) as tc:
        with (
            tc.tile_pool(name="wconst", bufs=1) as wpool,
            tc.tile_pool(name="sbresb", bufs=1) as sbresb,
            tc.tile_pool(name="sbrt", bufs=1) as sbrt,
            tc.tile_pool(name="sbh", bufs=1) as sbh,
            tc.tile_pool(name="sbrec", bufs=1) as sbrec,
            tc.tile_pool(name="psmm", bufs=1, space="PSUM") as psmm,
            tc.tile_pool(name="psl4", bufs=1, space="PSUM") as psl4,
        ):
            # DMA issue order: chunk-0 res + early weights first so compute
            # starts quickly; W4 stacks, recurrence weights and later res
            # chunks stream behind.
            res_sb0 = wpool.tile([128, 8 * RES_DIM], F32)
            nc.sync.dma_start(
                out=res_sb0[:],
                in_=res_d[0:T].rearrange("(p n) d -> p (n d)", p=128))
            wf = wpool.tile([128, FCOLS], F32)
            nc.sync.dma_start(out=wf[:], in_=wf_d[:])
            wa = wpool.tile([128, ACOLS], BF16)
            nc.sync.dma_start(out=wa[:], in_=wa_d[:])
            res_sb1 = wpool.tile([128, 3 * 8 * RES_DIM], F32)
            nc.sync.dma_start(
                out=res_sb1[:],
                in_=res_d[T:4 * T].rearrange("(p n) d -> p (n d)", p=128))
            wz = wpool.tile([128, ZCOLS], BF16)
            nc.sync.dma_start(out=wz[:], in_=wz_d[:])
            wr = wpool.tile([128, RCOLS], F32R)
            nc.sync.dma_start(out=wr[:], in_=wr_d[:])
            res_sb = wpool.tile([128, (NCH - 4) * 8 * RES_DIM], F32)
            nc.sync.dma_start(
                out=res_sb[:],
                in_=res_d[4 * T:].rearrange("(p n) d -> p (n d)", p=128))
            ident = wa[:, IDC:IDC + 128]

            raw_sb = sbrec.tile([96, T], F32)
            act_r = sbrec.tile([96, T], F32R)
            act_o = sbrec.tile([96, T], F32)
            raw_r = sbrec.tile([96, T], F32R)
            scr = sbrec.tile([1, 4], F32)
            scrD = sbrec.tile([1, 8], F32)

            pm4 = psl4.tile([96, T], F32)   # persistent raw accumulator

            pe_tail = None
            act_tail = None
            dve_tail = None

            def pe_touch(src_ap, dst_ap):
                """1x2 matmul on PE reading src (absorbing its producer's
                sem) and writing scratch cells at dst (PSUM, f32)."""
                nonlocal pe_tail
                m = nc.tensor.matmul(dst_ap, src_ap[:, 0:1], src_ap[:, 0:2],
                                     start=True, stop=True)
                _order(m, pe_tail)
                pe_tail = m
                return m

            tag_rr = [0]
            tag_state = [None] * MMB

            def new_mm_tile(name, width=T, dtype=F32, parts=128):
                """Allocate the next rotating psum tag tile. Pre-observes the
                tag's previous consumer with a touch matmul that writes into
                the tile itself (safe: the tile's real matmuls re-zero via
                start=True)."""
                tg = tag_rr[0] % MMB
                tag_rr[0] += 1
                st = tag_state[tg]
                if st is not None:
                    tw = psmm.tile([1, 2], F32, tag=f"mm{tg}", bufs=1,
                                   name=f"{name}_tw")
                    tile_, row_, col_ = st
                    pe_touch(tile_[row_:row_ + 1, col_:col_ + 2], tw[0:1, 0:2])
                    tag_state[tg] = None
                t = psmm.tile([parts, width], dtype, tag=f"mm{tg}", bufs=1,
                              name=name)
                return t, tg

            def mm(out_ap, lhs_ap, rhs_ap, **kw):
                nonlocal pe_tail
                m = nc.tensor.matmul(out_ap, lhs_ap, rhs_ap, **kw)
                _order(m, pe_tail)
                pe_tail = m
                return m

            def silu(out_ap, pm_ap, bias_ap, out_tile, tg, row, col,
                     func=AF.Silu):
                nonlocal act_tail
                s = nc.scalar.activation(out_ap, pm_ap, func, bias=bias_ap)
                _order(s, act_tail)
                act_tail = s
                if tg is not None:
                    tag_state[tg] = (out_tile, row, col)
                return s

            def dve(op):
                nonlocal dve_tail
                _order(op, dve_tail)
                dve_tail = op
                return op

            # Startup observes: PE on wa lane, ACT + DVE on wf lane.
            tw0 = psmm.tile([1, 2], F32, tag="mm0", bufs=1, name="warm")
            pe_touch(wa[0:1, 0:2], tw0[0:1, 0:2])
            nc.scalar.activation(scr[0:1, 0:1], wf[0:1, B1C:B1C + 1], AF.Copy)
            nc.vector.tensor_copy(scrD[0:1, 0:1], wf[0:1, B1C:B1C + 1])

            def res_src(i):
                if i == 0:
                    return res_sb0, 0
                if i < 4:
                    return res_sb1, (i - 1) * 8 * RES_DIM
                return res_sb, (i - 4) * 8 * RES_DIM

            def emit_conv(i):
                """DVE: convert chunk i's res slice fp32 -> bf16."""
                rq, coff = res_src(i)
                rb = sbresb.tile([128, 8 * RES_DIM], BF16, tag="rb", bufs=2,
                                 name="rb")
                dve(nc.vector.tensor_copy(
                    rb[:], rq[:, coff:coff + 8 * RES_DIM]))
                return rb

            def emit_tr(i, rb):
                """PE transposes (into a rotating tag slot) + DVE copy ->
                rT [100, 1024] bf16 in SBUF."""
                nonlocal pe_tail
                ptr, tgt = new_mm_tile("ptr", width=T, dtype=BF16,
                                       parts=RES_DIM)
                for nn_ in range(8):
                    t_ = nc.tensor.transpose(
                        ptr[:, nn_ * 128:(nn_ + 1) * 128],
                        rb[:, nn_ * RES_DIM:(nn_ + 1) * RES_DIM],
                        ident,
                    )
                    _order(t_, pe_tail)
                    pe_tail = t_
                rT = sbrt.tile([RES_DIM, T], BF16, tag="rT", bufs=3, name="rT")
                dve(nc.vector.tensor_copy(rT[:], ptr[:]))
                tag_state[tgt] = (rT, 0, 0)
                return rT

            rb_next = emit_conv(0)
            rt_next = emit_tr(0, rb_next)
            pending_l4 = []
            for i in range(NCH):
                rT = rt_next

                # L1: 6 chambers, one [128,1024] psum tile each
                h1s = []
                for cp3 in range(3):
                    ha = sbh.tile([128, T], BF16, tag="h1", bufs=6, name="h1a")
                    hb = sbh.tile([128, T], BF16, tag="h1", bufs=6, name="h1b")
                    pa, ta = new_mm_tile("pm1a")
                    pb, tb = new_mm_tile("pm1b")
                    ca, cb = 2 * cp3, 2 * cp3 + 1
                    for s in range(2):
                        mm(pa[:, s * 512:(s + 1) * 512],
                           wa[0:RES_DIM, W1C + ca * 128:W1C + (ca + 1) * 128],
                           rT[:, s * 512:(s + 1) * 512], start=True, stop=True)
                    for s in range(2):
                        mm(pb[:, s * 512:(s + 1) * 512],
                           wa[0:RES_DIM, W1C + cb * 128:W1C + (cb + 1) * 128],
                           rT[:, s * 512:(s + 1) * 512], start=True, stop=True)
                    silu(ha[:], pa[:], wf[:, B1C + ca:B1C + ca + 1],
                         ha, ta, 0, 0)
                    silu(hb[:], pb[:], wf[:, B1C + cb:B1C + cb + 1],
                         hb, tb, 0, 0)
                    h1s.extend([ha, hb])

                # next chunk's res conversion can start as soon as DVE is free
                if i + 1 < NCH:
                    rb_next = emit_conv(i + 1)

                # L2: 3 pairs, both chambers stacked on out partitions
                l2t = []
                for pr in range(3):
                    pm2, tg2 = new_mm_tile("pm2")
                    for s in range(2):
                        mm(pm2[:, s * 512:(s + 1) * 512],
                           wa[:, W2BC + pr * 128:W2BC + (pr + 1) * 128],
                           h1s[2 * pr + 1][:, s * 512:(s + 1) * 512],
                           start=True, stop=False)
                        mm(pm2[0:64, s * 512:(s + 1) * 512],
                           wa[:, W2C + 2 * pr * 64:W2C + (2 * pr + 1) * 64],
                           h1s[2 * pr][:, s * 512:(s + 1) * 512],
                           start=False, stop=True)
                    l2t.append((pm2, tg2))
                h2s = []
                for pr in range(3):
                    pm2, tg2 = l2t[pr]
                    h2 = sbh.tile([128, T], BF16, tag="h2", bufs=4, name="h2")
                    silu(h2[:], pm2[:], wf[:, B2PC + pr:B2PC + pr + 1],
                         h2, tg2, 0, 0)
                    h2s.append(h2)

                if i + 1 < NCH:
                    rt_next = emit_tr(i + 1, rb_next)
                if pending_l4:
                    pending_l4.pop(0)()

                # L3: pairs 0,1 merged into one [128,1024] tile; pair 2
                # sample-folded into [128,512]
                h3a = sbh.tile([128, T], BF16, tag="h3a", bufs=3, name="h3a")
                h3b = sbh.tile([128, 512], BF16, tag="h3b", bufs=3, name="h3b")
                pa3, ta3 = new_mm_tile("pm3")
                pc3, tc3 = new_mm_tile("pm3b", width=512)
                for s in range(2):
                    mm(pa3[:, s * 512:(s + 1) * 512],
                       wa[:, W3BC:W3BC + 128],
                       h2s[1][:, s * 512:(s + 1) * 512], start=True, stop=False)
                    mm(pa3[0:64, s * 512:(s + 1) * 512],
                       wa[:, W3AC:W3AC + 64],
                       h2s[0][:, s * 512:(s + 1) * 512], start=False, stop=True)
                mm(pc3[0:64, 0:512], wa[:, W3CC:W3CC + 64],
                   h2s[2][:, 0:512], start=True, stop=False)
                mm(pc3[64:128, 0:512], wa[:, W3CC:W3CC + 64],
                   h2s[2][:, 512:1024], start=False, stop=True)
                silu(h3a[:], pa3[:], wf[:, B3PC:B3PC + 1], h3a, ta3, 0, 0)
                silu(h3b[:], pc3[:], wf[:, B3P2C:B3P2C + 1], h3b, tc3, 0, 0)

                # L4 deferred past the next chunk's L1/L2: accumulate raw
                # rows 6i:6i+6 into the persistent [96,1024] psum tile.
                def emit_l4(i=i, h3a=h3a, h3b=h3b):
                    if i == 0:
                        # observe the W4-stack DMA lane; writing pm4 cells is
                        # safe: the first real matmul start=True re-zeroes
                        pe_touch(wz[0:1, 0:2], pm4[0:1, 0:2])
                    for s in range(2):
                        mm(pm4[0:96, s * 512:(s + 1) * 512],
                           wz[:, W4AC + 96 * i:W4AC + 96 * (i + 1)],
                           h3a[:, s * 512:(s + 1) * 512],
                           start=(i == 0), stop=False)
                        mm(pm4[0:96, s * 512:(s + 1) * 512],
                           wz[64 * s:64 * s + 64,
                              W4BC + 96 * i:W4BC + 96 * (i + 1)],
                           h3b[64 * s:64 * s + 64, 0:512],
                           start=False, stop=(i == NCH - 1))
                pending_l4.append(emit_l4)

            if pending_l4:
                pending_l4.pop(0)()

            # ---- tail: raw materialization + coupled sigmoid recurrence ----
            cpr = dve(nc.vector.tensor_scalar(
                out=raw_sb[:], in0=pm4[0:96, :],
                scalar1=wf[0:96, B4C:B4C + 1], scalar2=None, op0=ALU.add))
            dve(nc.vector.tensor_copy(raw_r[:], raw_sb[:]))
            nc.sync.dma_start(out=raw_d[:], in_=raw_sb[:])
            tw1, _ = new_mm_tile("warm2", width=4)
            pe_touch(wr[0:1, 0:2], tw1[0:1, 0:2])      # observe wr DMA lane
            pe_touch(raw_r[0:1, 0:2], tw1[0:1, 2:4])   # observe DVE raw_r
            silu(act_r[:], pm4[0:96, :], wf[0:96, B4C:B4C + 1],
                 None, None, 0, 0, func=AF.Sigmoid)
            for kk in range(CF_ITERS):
                dst = act_r if kk < CF_ITERS - 1 else act_o
                for s in range(2):
                    pm5, tg5 = new_mm_tile("pm5", width=512)
                    mm(pm5[0:96, 0:512],
                       wr[0:96, CDC:CDC + 96],
                       act_r[:, s * 512:(s + 1) * 512],
                       start=True, stop=False)
                    mm(pm5[0:96, 0:512],
                       wr[0:96, I96C:I96C + 96],
                       raw_r[:, s * 512:(s + 1) * 512],
                       start=False, stop=True)
                    silu(dst[:, s * 512:(s + 1) * 512], pm5[0:96, 0:512],
                         0.0, dst, tg5, 0, s * 512, func=AF.Sigmoid)
            nc.sync.dma_start(out=act_d[:], in_=act_o[:])

    return nc


def _pack_consts(W1, b1, W2, b2, W3, b3, W4, b4, coupling, decay):
    wf = np.zeros((128, FCOLS), dtype=np.float32)
    for c in range(6):
        wf[:, B1C + c] = b1[c]
    for pr in range(3):
        wf[0:64, B2PC + pr] = b2[2 * pr]
        wf[64:128, B2PC + pr] = b2[2 * pr + 1]
    for c in range(4):
        wf[c * 32:(c + 1) * 32, B3PC] = b3[c]
    for s in range(2):
        wf[64 * s:64 * s + 32, B3P2C] = b3[4]
        wf[64 * s + 32:64 * s + 64, B3P2C] = b3[5]
    wf[0:96, B4C] = np.tile(b4, 16)

    wr = np.zeros((128, RCOLS), dtype=np.float32)
    cd = (decay[:, None] * coupling * CF_K).astype(np.float32)
    for g in range(16):
        wr[6 * g:6 * g + 6, CDC + 6 * g:CDC + 6 * g + 6] = cd
    wr[0:96, I96C:I96C + 96] = np.eye(96, dtype=np.float32)

    wa = np.zeros((128, ACOLS), dtype=np.float32)
    wa[:, IDC:IDC + 128] = np.eye(128, dtype=np.float32)
    for c in range(6):
        wa[0:RES_DIM, W1C + c * 128:W1C + (c + 1) * 128] = W1[c]
        wa[0:128, W2C + c * 64:W2C + (c + 1) * 64] = W2[c]
    for pr in range(3):
        # odd chamber shifted to out rows 64:127; cols 0:64 stay zero so
        # start=True clears the even chamber's rows for the accumulate
        wa[:, W2BC + pr * 128 + 64:W2BC + (pr + 1) * 128] = W2[2 * pr + 1]
    wa[0:64, W3AC:W3AC + 32] = W3[0]
    wa[64:128, W3AC + 32:W3AC + 64] = W3[1]
    wa[0:64, W3BC + 64:W3BC + 96] = W3[2]
    wa[64:128, W3BC + 96:W3BC + 128] = W3[3]
    wa[0:64, W3CC:W3CC + 32] = W3[4]
    wa[64:128, W3CC + 32:W3CC + 64] = W3[5]

    wz = np.zeros((128, ZCOLS), dtype=np.float32)
    for i in range(16):
        ba = W4AC + 96 * i
        for c in range(4):
            wz[c * 32:(c + 1) * 32, ba + 6 * i + c] = W4[c]
        bb = W4BC + 96 * i
        for s in range(2):
            wz[64 * s:64 * s + 32, bb + 6 * i + 4] = W4[4]
            wz[64 * s + 32:64 * s + 64, bb + 6 * i + 5] = W4[5]
    return (wf, wr, wa.astype(ml_dtypes.bfloat16),
            wz.astype(ml_dtypes.bfloat16))


def _unshard(per_core, key):
    """[96, T] group layout -> [BS, 6] per core, concat to [B, 6].

    Chunk 0: sample p*8+n8. Chunks 1-3: 1024 + p*24 + (i-1)*8 + n8.
    Chunks 4-15: 4096 + p*96 + (i-4)*8 + n8."""
    outs = []
    for r in per_core:
        a = r[key].reshape(NCH, 6, 8, 128)             # [i, c, n8, p]
        out = np.empty((BS, 6), dtype=a.dtype)
        out[0:T] = a[0].transpose(2, 1, 0).reshape(T, 6)
        out[T:4 * T] = a[1:4].transpose(3, 0, 2, 1).reshape(3 * T, 6)
        out[4 * T:] = a[4:].transpose(3, 0, 2, 1).reshape(12 * T, 6)
        outs.append(out)
    return np.concatenate(outs, axis=0)


def kernel(res, W1, b1, W2, b2, W3, b3, W4, b4, coupling, decay):
    res = np.asarray(res, dtype=np.float32)
    args = [np.asarray(a, dtype=np.float32)
            for a in (W1, b1, W2, b2, W3, b3, W4, b4, coupling, decay)]
    wf, wr, wa, wz = _pack_consts(*args)

    nc = build_module()
    in_maps = [
        {"res": np.ascontiguousarray(res[i * BS:(i + 1) * BS]),
         "wf": wf, "wr": wr, "wa": wa, "wz": wz}
        for i in range(NCORES)
    ]
    results = run_bass_kernel_spmd(nc, in_maps, core_ids=list(range(NCORES)))
    act = _unshard(results.results, "act_out")
    raw = _unshard(results.results, "raw_out")
    return act, raw


# revision 16
# speedup vs baseline: 1.1816x; 1.0123x over previous
"""Trainium2 Bass kernel for nn_Chambers (6-tower MLP + coupled sigmoid recurrence).

Data-parallel over 8 NeuronCores: each core processes a 16384-sample shard in
16 chunks of 1024 samples. v2 design:

- bf16 matmul pipeline: res is converted fp32->bf16 on DVE, PE-transposed in
  bf16 (1.0 cycles/row vs 2.0 for fp32), and L1-L4 run as bf16 matmuls with
  chambers packed block-diagonally. All h-activations are bf16 in SBUF.
- L3 chamber pair (4,5) is sample-folded into a [128, 512] tile (samples
  0:511 on partitions 0:63, 512:1023 on 64:127) so its SiLU costs 512 ACT
  rows instead of 1024.
- L4 accumulates raw directly into a persistent [96, 1024] PSUM tile across
  all 16 chunks via per-chunk W4 column stacks (chunk i writes rows 6i:6i+5;
  other rows accumulate zeros). This removes the per-chunk raw bias-copy on
  ACT and all SWDGE assembly DMAs.
- PSUM: 3 rotating [128,1024] matmul tags (6 banks) + the persistent raw
  accumulator (2 banks). Transpose tiles ride the same tag rotation (bf16,
  half a slot); PE "touch" matmuls write into the tag tile being allocated,
  so no scratch bank is needed.
- The 5-step coupled sigmoid recurrence runs on the resident raw tile via a
  block-diagonal [96,96] f32r matmul.

Sync discipline (walrus: at most 1 sem wait + 1 update per instruction):
PSUM tag rotation is pre-observed by 1x2 "touch" matmuls on PE; all other
cross-engine deps resolve to a single auto-added wait because each
instruction's data wait subsumes its WAR wait on the same engine-sem lane.
"""
import numpy as np
import ml_dtypes

import concourse.bass as bass
import concourse.mybir as mybir
from concourse.bass_utils import run_bass_kernel_spmd
from concourse.tile import TileContext
from concourse.tile_scheduler import N_PROCS
from concourse.vector_clock import ScopedClock
from bass_rust import add_dep_helper

F32 = mybir.dt.float32
F32R = mybir.dt.float32r
BF16 = mybir.dt.bfloat16
AF = mybir.ActivationFunctionType
ALU = mybir.AluOpType

B = 131072
NCORES = 8
BS = B // NCORES           # 16384 samples per core
T = 1024                   # chunk (samples)
NCH = BS // T              # 16 chunks
RES_DIM = 100
CF_ITERS = 5
CF_K = 0.02

# wf (fp32) column layout: per-partition bias vectors
B1C = 0        # 6 cols (b1 per chamber, 128 rows)
B2PC = 6       # 3 cols (pair-packed b2: rows 0:64=b2[2pr], 64:128=b2[2pr+1])
B3PC = 9       # 1 col (b3 chambers 0-3 by 32s)
B3P2C = 10     # 1 col (b3 ch4,5 folded twice over 128 rows)
B4C = 11       # 1 col (b4 tiled x16 over 96 rows)
FCOLS = 12

# wr (fp32r): recurrence matrices
CDC = 0        # 96 (block-diag decay*coupling*k)
I96C = 96      # 96 (identity, for raw+delta accumulate)
RCOLS = 192

# wi (bf16): identity + L1 weights (first DMA, gates compute start)
IDC = 0                    # identity [128,128] for PE transpose
W1C = 128                  # 6*128
ICOLS = W1C + 6 * 128

# wa (bf16): L2/L3 weights
W2C = 0                    # 6*64
W2BC = W2C + 6 * 64        # 3*128: odd-chamber W2 shifted to out rows 64:127
W3AC = W2BC + 3 * 128      # 64: pair 0 (both chambers packed on K)
W3BC = W3AC + 64           # 128: pair 1 shifted to out rows 64:127
W3CC = W3BC + 128          # 64: pair 2
ACOLS = W3CC + 64

# wz (bf16): per-chunk W4 stacks
W4AC = 0                   # 16*96 (chambers 0-3)
W4BC = 16 * 96             # 16*96 (ch 4,5; both sample halves)
ZCOLS = 2 * 16 * 96


class TC(TileContext):
    """TileContext with a walrus-compatible epilogue (split final waits)."""

    def _drain_and_barrier(self, tick_clock, wait_clock):
        nc = self.nc
        full = ScopedClock({None: tick_clock.global_clock})
        for scope, vc in full.items():
            for proc in range(N_PROCS):
                t = vc.peek_next(proc) - 1
                if t > 0:
                    sc = ScopedClock()
                    sc.require_at_least(scope, proc, t)
                    w = nc.sync.nop(nofuse=True)
                    wait_clock.add_sem_waits(w.ins, sc)
        for eng in nc.engines.values():
            eng.drain(fusable=False)
        nc.all_engine_barrier(sem_only=True)
        assert self.sems is not None
        popped = nc._tile_sem_poison_stack.pop()
        assert popped is self._sem_poison
        nc.clear_and_free_semaphores(list(self.sems.allocated().values()))
        for eng in nc.engines.values():
            eng.drain(fusable=False)
        nc.all_engine_barrier(sem_only=True)


def _order(after_inst, before_inst):
    if after_inst is not None and before_inst is not None:
        add_dep_helper(after_inst.ins, before_inst.ins, sync=False, reason="order")


def build_module():
    nc = bass.Bass()
    res_d = nc.dram_tensor("res", [BS, RES_DIM], F32, kind="ExternalInput")
    wf_d = nc.dram_tensor("wf", [128, FCOLS], F32, kind="ExternalInput")
    wr_d = nc.dram_tensor("wr", [128, RCOLS], F32R, kind="ExternalInput")
    wi_d = nc.dram_tensor("wi", [128, ICOLS], BF16, kind="ExternalInput")
    wa_d = nc.dram_tensor("wa", [128, ACOLS], BF16, kind="ExternalInput")
    wz_d = nc.dram_tensor("wz", [128, ZCOLS], BF16, kind="ExternalInput")
    raw_d = nc.dram_tensor("raw_out", [96, T], F32, kind="ExternalOutput")
    act_d = nc.dram_tensor("act_out", [96, T], F32, kind="ExternalOutput")

    MMB = 3  # rotating matmul psum tags

    with TC(nc) as tc:
        with (
            tc.tile_pool(name="wconst", bufs=1) as wpool,
            tc.tile_pool(name="sbresb", bufs=1) as sbresb,
            tc.tile_pool(name="sbrt", bufs=1) as sbrt,
            tc.tile_pool(name="sbh", bufs=1) as sbh,
            tc.tile_pool(name="sbrec", bufs=1) as sbrec,
            tc.tile_pool(name="psmm", bufs=1, space="PSUM") as psmm,
            tc.tile_pool(name="psl4", bufs=1, space="PSUM") as psl4,
        ):
            # DMA issue order: transpose identity + W1 + chunk-0 res first so
            # compute starts quickly; L2/L3 weights next; W4 stacks,
            # recurrence weights and later res chunks stream behind.
            wi = wpool.tile([128, ICOLS], BF16)
            nc.sync.dma_start(out=wi[:], in_=wi_d[:])
            res_sb0 = wpool.tile([128, 8 * RES_DIM], F32)
            nc.sync.dma_start(
                out=res_sb0[:],
                in_=res_d[0:T].rearrange("(p n) d -> p (n d)", p=128))
            wf = wpool.tile([128, FCOLS], F32)
            nc.sync.dma_start(out=wf[:], in_=wf_d[:])
            wa = wpool.tile([128, ACOLS], BF16)
            nc.sync.dma_start(out=wa[:], in_=wa_d[:])
            res_sb1 = wpool.tile([128, 3 * 8 * RES_DIM], F32)
            nc.sync.dma_start(
                out=res_sb1[:],
                in_=res_d[T:4 * T].rearrange("(p n) d -> p (n d)", p=128))
            wz = wpool.tile([128, ZCOLS], BF16)
            nc.sync.dma_start(out=wz[:], in_=wz_d[:])
            wr = wpool.tile([128, RCOLS], F32R)
            nc.sync.dma_start(out=wr[:], in_=wr_d[:])
            res_sb = wpool.tile([128, (NCH - 4) * 8 * RES_DIM], F32)
            nc.sync.dma_start(
                out=res_sb[:],
                in_=res_d[4 * T:].rearrange("(p n) d -> p (n d)", p=128))
            ident = wi[:, IDC:IDC + 128]

            raw_sb = sbrec.tile([96, T], F32)
            act_r = sbrec.tile([96, T], F32R)
            act_o = sbrec.tile([96, T], F32)
            raw_r = sbrec.tile([96, T], F32R)
            scr = sbrec.tile([1, 4], F32)
            scrD = sbrec.tile([1, 8], F32)

            pm4 = psl4.tile([96, T], F32)   # persistent raw accumulator

            pe_tail = None
            act_tail = None
            dve_tail = None

            def pe_touch(src_ap, dst_ap):
                """1x2 matmul on PE reading src (absorbing its producer's
                sem) and writing scratch cells at dst (PSUM, f32)."""
                nonlocal pe_tail
                m = nc.tensor.matmul(dst_ap, src_ap[:, 0:1], src_ap[:, 0:2],
                                     start=True, stop=True)
                _order(m, pe_tail)
                pe_tail = m
                return m

            tag_rr = [0]
            tag_state = [None] * MMB

            def new_mm_tile(name, width=T, dtype=F32, parts=128):
                """Allocate the next rotating psum tag tile. Pre-observes the
                tag's previous consumer with a touch matmul that writes into
                the tile itself (safe: the tile's real matmuls re-zero via
                start=True)."""
                tg = tag_rr[0] % MMB
                tag_rr[0] += 1
                st = tag_state[tg]
                if st is not None:
                    tw = psmm.tile([1, 2], F32, tag=f"mm{tg}", bufs=1,
                                   name=f"{name}_tw")
                    tile_, row_, col_ = st
                    pe_touch(tile_[row_:row_ + 1, col_:col_ + 2], tw[0:1, 0:2])
                    tag_state[tg] = None
                t = psmm.tile([parts, width], dtype, tag=f"mm{tg}", bufs=1,
                              name=name)
                return t, tg

            def mm(out_ap, lhs_ap, rhs_ap, **kw):
                nonlocal pe_tail
                m = nc.tensor.matmul(out_ap, lhs_ap, rhs_ap, **kw)
                _order(m, pe_tail)
                pe_tail = m
                return m

            def silu(out_ap, pm_ap, bias_ap, out_tile, tg, row, col,
                     func=AF.Silu):
                nonlocal act_tail
                s = nc.scalar.activation(out_ap, pm_ap, func, bias=bias_ap)
                _order(s, act_tail)
                act_tail = s
                if tg is not None:
                    tag_state[tg] = (out_tile, row, col)
                return s

            def dve(op):
                nonlocal dve_tail
                _order(op, dve_tail)
                dve_tail = op
                return op

            # Startup observes: PE on wi lane, ACT + DVE on wf lane.
            tw0 = psmm.tile([1, 2], F32, tag="mm0", bufs=1, name="warm")
            pe_touch(wi[0:1, 0:2], tw0[0:1, 0:2])
            nc.scalar.activation(scr[0:1, 0:1], wf[0:1, B1C:B1C + 1], AF.Copy)
            nc.vector.tensor_copy(scrD[0:1, 0:1], wf[0:1, B1C:B1C + 1])

            def res_src(i):
                if i == 0:
                    return res_sb0, 0
                if i < 4:
                    return res_sb1, (i - 1) * 8 * RES_DIM
                return res_sb, (i - 4) * 8 * RES_DIM

            def emit_conv(i):
                """DVE: convert chunk i's res slice fp32 -> bf16."""
                rq, coff = res_src(i)
                rb = sbresb.tile([128, 8 * RES_DIM], BF16, tag="rb", bufs=2,
                                 name="rb")
                dve(nc.vector.tensor_copy(
                    rb[:], rq[:, coff:coff + 8 * RES_DIM]))
                return rb

            def emit_tr(i, rb):
                """PE transposes (into a rotating tag slot) + DVE copy ->
                rT [100, 1024] bf16 in SBUF."""
                nonlocal pe_tail
                ptr, tgt = new_mm_tile("ptr", width=T, dtype=BF16,
                                       parts=RES_DIM)
                for nn_ in range(8):
                    t_ = nc.tensor.transpose(
                        ptr[:, nn_ * 128:(nn_ + 1) * 128],
                        rb[:, nn_ * RES_DIM:(nn_ + 1) * RES_DIM],
                        ident,
                    )
                    _order(t_, pe_tail)
                    pe_tail = t_
                rT = sbrt.tile([RES_DIM, T], BF16, tag="rT", bufs=3, name="rT")
                dve(nc.vector.tensor_copy(rT[:], ptr[:]))
                tag_state[tgt] = (rT, 0, 0)
                return rT

            rb_next = emit_conv(0)
            rt_next = emit_tr(0, rb_next)
            pending_l4 = []
            for i in range(NCH):
                rT = rt_next

                # L1: 6 chambers, one [128,1024] psum tile each
                h1s = []
                for cp3 in range(3):
                    ha = sbh.tile([128, T], BF16, tag="h1", bufs=6, name="h1a")
                    hb = sbh.tile([128, T], BF16, tag="h1", bufs=6, name="h1b")
                    pa, ta = new_mm_tile("pm1a")
                    pb, tb = new_mm_tile("pm1b")
                    ca, cb = 2 * cp3, 2 * cp3 + 1
                    for s in range(2):
                        mm(pa[:, s * 512:(s + 1) * 512],
                           wi[0:RES_DIM, W1C + ca * 128:W1C + (ca + 1) * 128],
                           rT[:, s * 512:(s + 1) * 512], start=True, stop=True)
                    for s in range(2):
                        mm(pb[:, s * 512:(s + 1) * 512],
                           wi[0:RES_DIM, W1C + cb * 128:W1C + (cb + 1) * 128],
                           rT[:, s * 512:(s + 1) * 512], start=True, stop=True)
                    silu(ha[:], pa[:], wf[:, B1C + ca:B1C + ca + 1],
                         ha, ta, 0, 0)
                    silu(hb[:], pb[:], wf[:, B1C + cb:B1C + cb + 1],
                         hb, tb, 0, 0)
                    h1s.extend([ha, hb])

                # next chunk's res conversion can start as soon as DVE is free
                if i + 1 < NCH:
                    rb_next = emit_conv(i + 1)

                # L2: 3 pairs, both chambers stacked on out partitions
                if i == 0:
                    # one-time observe of the wa DMA lane so the first L2
                    # matmul carries only its h1 data wait; pm4 cells are
                    # safe scratch until L4(0) opens the accumulation group
                    pe_touch(wa[0:1, 0:2], pm4[0:1, 2:4])
                l2t = []
                for pr in range(3):
                    pm2, tg2 = new_mm_tile("pm2")
                    for s in range(2):
                        mm(pm2[:, s * 512:(s + 1) * 512],
                           wa[:, W2BC + pr * 128:W2BC + (pr + 1) * 128],
                           h1s[2 * pr + 1][:, s * 512:(s + 1) * 512],
                           start=True, stop=False)
                        mm(pm2[0:64, s * 512:(s + 1) * 512],
                           wa[:, W2C + 2 * pr * 64:W2C + (2 * pr + 1) * 64],
                           h1s[2 * pr][:, s * 512:(s + 1) * 512],
                           start=False, stop=True)
                    l2t.append((pm2, tg2))
                h2s = []
                for pr in range(3):
                    pm2, tg2 = l2t[pr]
                    h2 = sbh.tile([128, T], BF16, tag="h2", bufs=4, name="h2")
                    silu(h2[:], pm2[:], wf[:, B2PC + pr:B2PC + pr + 1],
                         h2, tg2, 0, 0)
                    h2s.append(h2)

                if i + 1 < NCH:
                    rt_next = emit_tr(i + 1, rb_next)

                # L3: pairs 0,1 merged into one [128,1024] tile; pair 2
                # sample-folded into [128,512]
                h3a = sbh.tile([128, T], BF16, tag="h3a", bufs=3, name="h3a")
                h3b = sbh.tile([128, 512], BF16, tag="h3b", bufs=3, name="h3b")
                pa3, ta3 = new_mm_tile("pm3")
                pc3, tc3 = new_mm_tile("pm3b", width=512)
                for s in range(2):
                    mm(pa3[:, s * 512:(s + 1) * 512],
                       wa[:, W3BC:W3BC + 128],
                       h2s[1][:, s * 512:(s + 1) * 512], start=True, stop=False)
                    mm(pa3[0:64, s * 512:(s + 1) * 512],
                       wa[:, W3AC:W3AC + 64],
                       h2s[0][:, s * 512:(s + 1) * 512], start=False, stop=True)
                mm(pc3[0:64, 0:512], wa[:, W3CC:W3CC + 64],
                   h2s[2][:, 0:512], start=True, stop=False)
                mm(pc3[64:128, 0:512], wa[:, W3CC:W3CC + 64],
                   h2s[2][:, 512:1024], start=False, stop=True)
                silu(h3a[:], pa3[:], wf[:, B3PC:B3PC + 1], h3a, ta3, 0, 0)
                silu(h3b[:], pc3[:], wf[:, B3P2C:B3P2C + 1], h3b, tc3, 0, 0)
                if pending_l4:
                    pending_l4.pop(0)()

                # L4 deferred past the next chunk's L1/L2: accumulate raw
                # rows 6i:6i+6 into the persistent [96,1024] psum tile.
                def emit_l4(i=i, h3a=h3a, h3b=h3b):
                    if i == 0:
                        # observe the W4-stack DMA lane; writing pm4 cells is
                        # safe: the first real matmul start=True re-zeroes
                        pe_touch(wz[0:1, 0:2], pm4[0:1, 0:2])
                    for s in range(2):
                        mm(pm4[0:96, s * 512:(s + 1) * 512],
                           wz[:, W4AC + 96 * i:W4AC + 96 * (i + 1)],
                           h3a[:, s * 512:(s + 1) * 512],
                           start=(i == 0), stop=False)
                        mm(pm4[0:96, s * 512:(s + 1) * 512],
                           wz[64 * s:64 * s + 64,
                              W4BC + 96 * i:W4BC + 96 * (i + 1)],
                           h3b[64 * s:64 * s + 64, 0:512],
                           start=False, stop=(i == NCH - 1))
                pending_l4.append(emit_l4)

            if pending_l4:
                pending_l4.pop(0)()

            # ---- tail: raw materialization + coupled sigmoid recurrence ----
            cpr = dve(nc.vector.tensor_scalar(
                out=raw_sb[:], in0=pm4[0:96, :],
                scalar1=wf[0:96, B4C:B4C + 1], scalar2=None, op0=ALU.add))
            dve(nc.vector.tensor_copy(raw_r[:], raw_sb[:]))
            nc.sync.dma_start(out=raw_d[:], in_=raw_sb[:])
            tw1, _ = new_mm_tile("warm2", width=4)
            pe_touch(wr[0:1, 0:2], tw1[0:1, 0:2])      # observe wr DMA lane
            pe_touch(raw_r[0:1, 0:2], tw1[0:1, 2:4])   # observe DVE raw_r
            silu(act_r[:], pm4[0:96, :], wf[0:96, B4C:B4C + 1],
                 None, None, 0, 0, func=AF.Sigmoid)
            NQ = 4
            QW = T // NQ
            for kk in range(CF_ITERS):
                dst = act_r if kk < CF_ITERS - 1 else act_o
                for q in range(NQ):
                    lo, hi = q * QW, (q + 1) * QW
                    pm5, tg5 = new_mm_tile("pm5", width=QW)
                    mm(pm5[0:96, 0:QW],
                       wr[0:96, CDC:CDC + 96],
                       act_r[:, lo:hi], start=True, stop=False)
                    mm(pm5[0:96, 0:QW],
                       wr[0:96, I96C:I96C + 96],
                       raw_r[:, lo:hi], start=False, stop=True)
                    silu(dst[:, lo:hi], pm5[0:96, 0:QW],
                         0.0, dst, tg5, 0, lo, func=AF.Sigmoid)
                    if kk == CF_ITERS - 1:
                        nc.sync.dma_start(out=act_d[:, lo:hi],
                                          in_=act_o[:, lo:hi])

    return nc


def _pack_consts(W1, b1, W2, b2, W3, b3, W4, b4, coupling, decay):
    wf = np.zeros((128, FCOLS), dtype=np.float32)
    for c in range(6):
        wf[:, B1C + c] = b1[c]
    for pr in range(3):
        wf[0:64, B2PC + pr] = b2[2 * pr]
        wf[64:128, B2PC + pr] = b2[2 * pr + 1]
    for c in range(4):
        wf[c * 32:(c + 1) * 32, B3PC] = b3[c]
    for s in range(2):
        wf[64 * s:64 * s + 32, B3P2C] = b3[4]
        wf[64 * s + 32:64 * s + 64, B3P2C] = b3[5]
    wf[0:96, B4C] = np.tile(b4, 16)

    wr = np.zeros((128, RCOLS), dtype=np.float32)
    cd = (decay[:, None] * coupling * CF_K).astype(np.float32)
    for g in range(16):
        wr[6 * g:6 * g + 6, CDC + 6 * g:CDC + 6 * g + 6] = cd
    wr[0:96, I96C:I96C + 96] = np.eye(96, dtype=np.float32)

    wi = np.zeros((128, ICOLS), dtype=np.float32)
    wi[:, IDC:IDC + 128] = np.eye(128, dtype=np.float32)
    for c in range(6):
        wi[0:RES_DIM, W1C + c * 128:W1C + (c + 1) * 128] = W1[c]

    wa = np.zeros((128, ACOLS), dtype=np.float32)
    for c in range(6):
        wa[0:128, W2C + c * 64:W2C + (c + 1) * 64] = W2[c]
    for pr in range(3):
        # odd chamber shifted to out rows 64:127; cols 0:64 stay zero so
        # start=True clears the even chamber's rows for the accumulate
        wa[:, W2BC + pr * 128 + 64:W2BC + (pr + 1) * 128] = W2[2 * pr + 1]
    wa[0:64, W3AC:W3AC + 32] = W3[0]
    wa[64:128, W3AC + 32:W3AC + 64] = W3[1]
    wa[0:64, W3BC + 64:W3BC + 96] = W3[2]
    wa[64:128, W3BC + 96:W3BC + 128] = W3[3]
    wa[0:64, W3CC:W3CC + 32] = W3[4]
    wa[64:128, W3CC + 32:W3CC + 64] = W3[5]

    wz = np.zeros((128, ZCOLS), dtype=np.float32)
    for i in range(16):
        ba = W4AC + 96 * i
        for c in range(4):
            wz[c * 32:(c + 1) * 32, ba + 6 * i + c] = W4[c]
        bb = W4BC + 96 * i
        for s in range(2):
            wz[64 * s:64 * s + 32, bb + 6 * i + 4] = W4[4]
            wz[64 * s + 32:64 * s + 64, bb + 6 * i + 5] = W4[5]
    return (wf, wr, wi.astype(ml_dtypes.bfloat16),
            wa.astype(ml_dtypes.bfloat16), wz.astype(ml_dtypes.bfloat16))


def _unshard(per_core, key):
    """[96, T] group layout -> [BS, 6] per core, concat to [B, 6].

    Chunk 0: sample p*8+n8. Chunks 1-3: 1024 + p*24 + (i-1)*8 + n8.
    Chunks 4-15: 4096 + p*96 + (i-4)*8 + n8."""
    outs = []
    for r in per_core:
        a = r[key].reshape(NCH, 6, 8, 128)             # [i, c, n8, p]
        out = np.empty((BS, 6), dtype=a.dtype)
        out[0:T] = a[0].transpose(2, 1, 0).reshape(T, 6)
        out[T:4 * T] = a[1:4].transpose(3, 0, 2, 1).reshape(3 * T, 6)
        out[4 * T:] = a[4:].transpose(3, 0, 2, 1).reshape(12 * T, 6)
        outs.append(out)
    return np.concatenate(outs, axis=0)


def kernel(res, W1, b1, W2, b2, W3, b3, W4, b4, coupling, decay):
    res = np.asarray(res, dtype=np.float32)
    args = [np.asarray(a, dtype=np.float32)
            for a in (W1, b1, W2, b2, W3, b3, W4, b4, coupling, decay)]
    wf, wr, wi, wa, wz = _pack_consts(*args)

    nc = build_module()
    in_maps = [
        {"res": np.ascontiguousarray(res[i * BS:(i + 1) * BS]),
         "wf": wf, "wr": wr, "wi": wi, "wa": wa, "wz": wz}
        for i in range(NCORES)
    ]
    results = run_bass_kernel_spmd(nc, in_maps, core_ids=list(range(NCORES)))
    act = _unshard(results.results, "act_out")
    raw = _unshard(results.results, "raw_out")
    return act, raw


# revision 17
# speedup vs baseline: 1.2307x; 1.0416x over previous
"""Trainium2 Bass kernel for nn_Chambers (6-tower MLP + coupled sigmoid recurrence).

Data-parallel over 8 NeuronCores: each core processes a 16384-sample shard in
16 chunks of 1024 samples. v2 design:

- bf16 matmul pipeline: res is converted fp32->bf16 on DVE, PE-transposed in
  bf16 (1.0 cycles/row vs 2.0 for fp32), and L1-L4 run as bf16 matmuls with
  chambers packed block-diagonally. All h-activations are bf16 in SBUF.
- L3 chamber pair (4,5) is sample-folded into a [128, 512] tile (samples
  0:511 on partitions 0:63, 512:1023 on 64:127) so its SiLU costs 512 ACT
  rows instead of 1024.
- L4 accumulates raw directly into a persistent [96, 1024] PSUM tile across
  all 16 chunks via per-chunk W4 column stacks (chunk i writes rows 6i:6i+5;
  other rows accumulate zeros). This removes the per-chunk raw bias-copy on
  ACT and all SWDGE assembly DMAs.
- PSUM: 3 rotating [128,1024] matmul tags (6 banks) + the persistent raw
  accumulator (2 banks). Transpose tiles ride the same tag rotation (bf16,
  half a slot); PE "touch" matmuls write into the tag tile being allocated,
  so no scratch bank is needed.
- The 5-step coupled sigmoid recurrence runs on the resident raw tile via a
  block-diagonal [96,96] f32r matmul.

Sync discipline (walrus: at most 1 sem wait + 1 update per instruction):
PSUM tag rotation is pre-observed by 1x2 "touch" matmuls on PE; all other
cross-engine deps resolve to a single auto-added wait because each
instruction's data wait subsumes its WAR wait on the same engine-sem lane.
"""
import numpy as np
import ml_dtypes

import concourse.bass as bass
import concourse.mybir as mybir
from concourse.bass_utils import run_bass_kernel_spmd
from concourse.tile import TileContext
from concourse.tile_scheduler import N_PROCS
from concourse.vector_clock import ScopedClock
from bass_rust import add_dep_helper

F32 = mybir.dt.float32
F32R = mybir.dt.float32r
BF16 = mybir.dt.bfloat16
AF = mybir.ActivationFunctionType
ALU = mybir.AluOpType

B = 131072
NCORES = 8
BS = B // NCORES           # 16384 samples per core
T = 1024                   # chunk (samples)
NCH = BS // T              # 16 chunks
RES_DIM = 100
CF_ITERS = 5
CF_K = 0.02

# wf (fp32) column layout: per-partition bias vectors
B1C = 0        # 6 cols (b1 per chamber, 128 rows)
B2PC = 6       # 3 cols (pair-packed b2: rows 0:64=b2[2pr], 64:128=b2[2pr+1])
B3PC = 9       # 1 col (b3 chambers 0-3 by 32s)
B3P2C = 10     # 1 col (b3 ch4,5 folded twice over 128 rows)
B4C = 11       # 1 col (b4 tiled x16 over 96 rows)
FCOLS = 12

# wr (fp32r): recurrence matrices
CDC = 0        # 96 (block-diag decay*coupling*k)
I96C = 96      # 96 (identity, for raw+delta accumulate)
RCOLS = 192

# wi (bf16): identity + L1 weights (first DMA, gates compute start)
IDC = 0                    # identity [128,128] for PE transpose
W1C = 128                  # 6*128
ICOLS = W1C + 6 * 128

# wa (bf16): L2/L3 weights
W2C = 0                    # 6*64
W2BC = W2C + 6 * 64        # 3*128: odd-chamber W2 shifted to out rows 64:127
W3AC = W2BC + 3 * 128      # 64: pair 0 (both chambers packed on K)
W3BC = W3AC + 64           # 128: pair 1 shifted to out rows 64:127
W3CC = W3BC + 128          # 64: pair 2
ACOLS = W3CC + 64

# wz (bf16): per-chunk W4 stacks
W4AC = 0                   # 16*96 (chambers 0-3)
W4BC = 16 * 96             # 16*96 (ch 4,5; both sample halves)
ZCOLS = 2 * 16 * 96


class TC(TileContext):
    """TileContext with a walrus-compatible epilogue (split final waits)."""

    def _drain_and_barrier(self, tick_clock, wait_clock):
        nc = self.nc
        full = ScopedClock({None: tick_clock.global_clock})
        for scope, vc in full.items():
            for proc in range(N_PROCS):
                t = vc.peek_next(proc) - 1
                if t > 0:
                    sc = ScopedClock()
                    sc.require_at_least(scope, proc, t)
                    w = nc.sync.nop(nofuse=True)
                    wait_clock.add_sem_waits(w.ins, sc)
        for eng in nc.engines.values():
            eng.drain(fusable=False)
        nc.all_engine_barrier(sem_only=True)
        assert self.sems is not None
        popped = nc._tile_sem_poison_stack.pop()
        assert popped is self._sem_poison
        nc.clear_and_free_semaphores(list(self.sems.allocated().values()))
        for eng in nc.engines.values():
            eng.drain(fusable=False)
        nc.all_engine_barrier(sem_only=True)


def _order(after_inst, before_inst):
    if after_inst is not None and before_inst is not None:
        add_dep_helper(after_inst.ins, before_inst.ins, sync=False, reason="order")


def build_module():
    nc = bass.Bass()
    res_d = nc.dram_tensor("res", [BS, RES_DIM], F32, kind="ExternalInput")
    wf_d = nc.dram_tensor("wf", [128, FCOLS], F32, kind="ExternalInput")
    wr_d = nc.dram_tensor("wr", [128, RCOLS], F32R, kind="ExternalInput")
    wi_d = nc.dram_tensor("wi", [128, ICOLS], BF16, kind="ExternalInput")
    wa_d = nc.dram_tensor("wa", [128, ACOLS], BF16, kind="ExternalInput")
    wz_d = nc.dram_tensor("wz", [128, ZCOLS], BF16, kind="ExternalInput")
    raw_d = nc.dram_tensor("raw_out", [96, T], F32, kind="ExternalOutput")
    act_d = nc.dram_tensor("act_out", [96, T], F32, kind="ExternalOutput")

    MMB = 3  # rotating matmul psum tags

    with TC(nc) as tc:
        with (
            tc.tile_pool(name="wconst", bufs=1) as wpool,
            tc.tile_pool(name="sbresb", bufs=1) as sbresb,
            tc.tile_pool(name="sbrt", bufs=1) as sbrt,
            tc.tile_pool(name="sbh", bufs=1) as sbh,
            tc.tile_pool(name="sbrec", bufs=1) as sbrec,
            tc.tile_pool(name="psmm", bufs=1, space="PSUM") as psmm,
            tc.tile_pool(name="psl4", bufs=1, space="PSUM") as psl4,
        ):
            # DMA issue order: transpose identity + W1 + chunk-0 res first so
            # compute starts quickly; L2/L3 weights next; W4 stacks,
            # recurrence weights and later res chunks stream behind.
            wi = wpool.tile([128, ICOLS], BF16)
            nc.sync.dma_start(out=wi[:], in_=wi_d[:])
            res_sb0 = wpool.tile([128, 8 * RES_DIM], F32)
            nc.sync.dma_start(
                out=res_sb0[:],
                in_=res_d[0:T].rearrange("(p n) d -> p (n d)", p=128))
            wf = wpool.tile([128, FCOLS], F32)
            nc.sync.dma_start(out=wf[:], in_=wf_d[:])
            wa = wpool.tile([128, ACOLS], BF16)
            nc.sync.dma_start(out=wa[:], in_=wa_d[:])
            res_sb1 = wpool.tile([128, 3 * 8 * RES_DIM], F32)
            nc.sync.dma_start(
                out=res_sb1[:],
                in_=res_d[T:4 * T].rearrange("(p n) d -> p (n d)", p=128))
            wz = wpool.tile([128, ZCOLS], BF16)
            nc.sync.dma_start(out=wz[:], in_=wz_d[:])
            wr = wpool.tile([128, RCOLS], F32R)
            nc.sync.dma_start(out=wr[:], in_=wr_d[:])
            res_sb = wpool.tile([128, (NCH - 4) * 8 * RES_DIM], F32)
            nc.sync.dma_start(
                out=res_sb[:],
                in_=res_d[4 * T:].rearrange("(p n) d -> p (n d)", p=128))
            ident = wi[:, IDC:IDC + 128]

            raw_sb = sbrec.tile([96, T], F32)
            act_r = sbrec.tile([96, T], F32R)
            act_o = sbrec.tile([96, T], F32)
            raw_r = sbrec.tile([96, T], F32R)
            scr = sbrec.tile([1, 4], F32)
            scrD = sbrec.tile([1, 8], F32)

            pm4 = psl4.tile([96, T], F32)   # persistent raw accumulator

            pe_tail = None
            act_tail = None
            dve_tail = None

            def pe_touch(src_ap, dst_ap):
                """1x2 matmul on PE reading src (absorbing its producer's
                sem) and writing scratch cells at dst (PSUM, f32)."""
                nonlocal pe_tail
                m = nc.tensor.matmul(dst_ap, src_ap[:, 0:1], src_ap[:, 0:2],
                                     start=True, stop=True)
                _order(m, pe_tail)
                pe_tail = m
                return m

            tag_rr = [0]
            tag_state = [None] * MMB

            def new_mm_tile(name, width=T, dtype=F32, parts=128):
                """Allocate the next rotating psum tag tile. Pre-observes the
                tag's previous consumer with a touch matmul that writes into
                the tile itself (safe: the tile's real matmuls re-zero via
                start=True)."""
                tg = tag_rr[0] % MMB
                tag_rr[0] += 1
                st = tag_state[tg]
                if st is not None:
                    tw = psmm.tile([1, 2], F32, tag=f"mm{tg}", bufs=1,
                                   name=f"{name}_tw")
                    tile_, row_, col_ = st
                    pe_touch(tile_[row_:row_ + 1, col_:col_ + 2], tw[0:1, 0:2])
                    tag_state[tg] = None
                t = psmm.tile([parts, width], dtype, tag=f"mm{tg}", bufs=1,
                              name=name)
                return t, tg

            def mm(out_ap, lhs_ap, rhs_ap, **kw):
                nonlocal pe_tail
                m = nc.tensor.matmul(out_ap, lhs_ap, rhs_ap, **kw)
                _order(m, pe_tail)
                pe_tail = m
                return m

            def silu(out_ap, pm_ap, bias_ap, out_tile, tg, row, col,
                     func=AF.Silu):
                nonlocal act_tail
                s = nc.scalar.activation(out_ap, pm_ap, func, bias=bias_ap)
                _order(s, act_tail)
                act_tail = s
                if tg is not None:
                    tag_state[tg] = (out_tile, row, col)
                return s

            def dve(op):
                nonlocal dve_tail
                _order(op, dve_tail)
                dve_tail = op
                return op

            # Startup observes: PE on wi lane, ACT + DVE on wf lane.
            tw0 = psmm.tile([1, 2], F32, tag="mm0", bufs=1, name="warm")
            pe_touch(wi[0:1, 0:2], tw0[0:1, 0:2])
            nc.scalar.activation(scr[0:1, 0:1], wf[0:1, B1C:B1C + 1], AF.Copy)
            nc.vector.tensor_copy(scrD[0:1, 0:1], wf[0:1, B1C:B1C + 1])

            def res_src(i):
                if i == 0:
                    return res_sb0, 0
                if i < 4:
                    return res_sb1, (i - 1) * 8 * RES_DIM
                return res_sb, (i - 4) * 8 * RES_DIM

            def emit_conv(i):
                """DVE: convert chunk i's res slice fp32 -> bf16."""
                rq, coff = res_src(i)
                rb = sbresb.tile([128, 8 * RES_DIM], BF16, tag="rb", bufs=2,
                                 name="rb")
                dve(nc.vector.tensor_copy(
                    rb[:], rq[:, coff:coff + 8 * RES_DIM]))
                return rb

            def emit_tr(i, rb):
                """PE transposes (into a rotating tag slot) + DVE copy ->
                rT [100, 1024] bf16 in SBUF."""
                nonlocal pe_tail
                ptr, tgt = new_mm_tile("ptr", width=T, dtype=BF16,
                                       parts=RES_DIM)
                for nn_ in range(8):
                    t_ = nc.tensor.transpose(
                        ptr[:, nn_ * 128:(nn_ + 1) * 128],
                        rb[:, nn_ * RES_DIM:(nn_ + 1) * RES_DIM],
                        ident,
                    )
                    _order(t_, pe_tail)
                    pe_tail = t_
                rT = sbrt.tile([RES_DIM, T], BF16, tag="rT", bufs=3, name="rT")
                dve(nc.vector.tensor_copy(rT[:], ptr[:]))
                tag_state[tgt] = (rT, 0, 0)
                return rT

            rb_next = emit_conv(0)
            rt_next = emit_tr(0, rb_next)
            pending_l4 = []
            for i in range(NCH):
                rT = rt_next

                # L1: 6 chambers, one [128,1024] psum tile each
                h1s = []
                for cp3 in range(3):
                    ha = sbh.tile([128, T], BF16, tag="h1", bufs=6, name="h1a")
                    hb = sbh.tile([128, T], BF16, tag="h1", bufs=6, name="h1b")
                    pa, ta = new_mm_tile("pm1a")
                    pb, tb = new_mm_tile("pm1b")
                    ca, cb = 2 * cp3, 2 * cp3 + 1
                    for s in range(2):
                        mm(pa[:, s * 512:(s + 1) * 512],
                           wi[0:RES_DIM, W1C + ca * 128:W1C + (ca + 1) * 128],
                           rT[:, s * 512:(s + 1) * 512], start=True, stop=True)
                    for s in range(2):
                        mm(pb[:, s * 512:(s + 1) * 512],
                           wi[0:RES_DIM, W1C + cb * 128:W1C + (cb + 1) * 128],
                           rT[:, s * 512:(s + 1) * 512], start=True, stop=True)
                    silu(ha[:], pa[:], wf[:, B1C + ca:B1C + ca + 1],
                         ha, ta, 0, 0)
                    silu(hb[:], pb[:], wf[:, B1C + cb:B1C + cb + 1],
                         hb, tb, 0, 0)
                    h1s.extend([ha, hb])

                # next chunk's res conversion can start as soon as DVE is free
                if i + 1 < NCH:
                    rb_next = emit_conv(i + 1)

                # L2: 3 pairs, both chambers stacked on out partitions
                if i == 0:
                    # one-time observe of the wa DMA lane so the first L2
                    # matmul carries only its h1 data wait; pm4 cells are
                    # safe scratch until L4(0) opens the accumulation group
                    pe_touch(wa[0:1, 0:2], pm4[0:1, 2:4])
                l2t = []
                for pr in range(3):
                    pm2, tg2 = new_mm_tile("pm2")
                    for s in range(2):
                        mm(pm2[:, s * 512:(s + 1) * 512],
                           wa[:, W2BC + pr * 128:W2BC + (pr + 1) * 128],
                           h1s[2 * pr + 1][:, s * 512:(s + 1) * 512],
                           start=True, stop=False)
                        mm(pm2[0:64, s * 512:(s + 1) * 512],
                           wa[:, W2C + 2 * pr * 64:W2C + (2 * pr + 1) * 64],
                           h1s[2 * pr][:, s * 512:(s + 1) * 512],
                           start=False, stop=True)
                    l2t.append((pm2, tg2))
                h2s = []
                for pr in range(3):
                    pm2, tg2 = l2t[pr]
                    h2 = sbh.tile([128, T], BF16, tag="h2", bufs=4, name="h2")
                    silu(h2[:], pm2[:], wf[:, B2PC + pr:B2PC + pr + 1],
                         h2, tg2, 0, 0)
                    h2s.append(h2)

                if i + 1 < NCH:
                    rt_next = emit_tr(i + 1, rb_next)

                # L3: pairs 0,1 merged into one [128,1024] tile; pair 2
                # sample-folded into [128,512]
                h3a = sbh.tile([128, T], BF16, tag="h3a", bufs=3, name="h3a")
                h3b = sbh.tile([128, 512], BF16, tag="h3b", bufs=3, name="h3b")
                pa3, ta3 = new_mm_tile("pm3")
                for s in range(2):
                    mm(pa3[:, s * 512:(s + 1) * 512],
                       wa[:, W3BC:W3BC + 128],
                       h2s[1][:, s * 512:(s + 1) * 512], start=True, stop=False)
                    mm(pa3[0:64, s * 512:(s + 1) * 512],
                       wa[:, W3AC:W3AC + 64],
                       h2s[0][:, s * 512:(s + 1) * 512], start=False, stop=True)
                pc3, tc3 = new_mm_tile("pm3b", width=512)
                mm(pc3[0:64, 0:512], wa[:, W3CC:W3CC + 64],
                   h2s[2][:, 0:512], start=True, stop=False)
                mm(pc3[64:128, 0:512], wa[:, W3CC:W3CC + 64],
                   h2s[2][:, 512:1024], start=False, stop=True)
                silu(h3a[:], pa3[:], wf[:, B3PC:B3PC + 1], h3a, ta3, 0, 0)
                silu(h3b[:], pc3[:], wf[:, B3P2C:B3P2C + 1], h3b, tc3, 0, 0)
                if pending_l4:
                    pending_l4.pop(0)()

                # L4 deferred past the next chunk's L1/L2: accumulate raw
                # rows 6i:6i+6 into the persistent [96,1024] psum tile.
                def emit_l4(i=i, h3a=h3a, h3b=h3b):
                    if i == 0:
                        # observe the W4-stack DMA lane; writing pm4 cells is
                        # safe: the first real matmul start=True re-zeroes
                        pe_touch(wz[0:1, 0:2], pm4[0:1, 0:2])
                    for s in range(2):
                        mm(pm4[0:96, s * 512:(s + 1) * 512],
                           wz[:, W4AC + 96 * i:W4AC + 96 * (i + 1)],
                           h3a[:, s * 512:(s + 1) * 512],
                           start=(i == 0), stop=False)
                        mm(pm4[0:96, s * 512:(s + 1) * 512],
                           wz[64 * s:64 * s + 64,
                              W4BC + 96 * i:W4BC + 96 * (i + 1)],
                           h3b[64 * s:64 * s + 64, 0:512],
                           start=False, stop=(i == NCH - 1))
                pending_l4.append(emit_l4)

            if pending_l4:
                pending_l4.pop(0)()

            # ---- tail: raw materialization + coupled sigmoid recurrence ----
            cpr = dve(nc.vector.tensor_scalar(
                out=raw_sb[:], in0=pm4[0:96, :],
                scalar1=wf[0:96, B4C:B4C + 1], scalar2=None, op0=ALU.add))
            dve(nc.vector.tensor_copy(raw_r[:], raw_sb[:]))
            nc.sync.dma_start(out=raw_d[:], in_=raw_sb[:])
            tw1, _ = new_mm_tile("warm2", width=4)
            pe_touch(wr[0:1, 0:2], tw1[0:1, 0:2])      # observe wr DMA lane
            pe_touch(raw_r[0:1, 0:2], tw1[0:1, 2:4])   # observe DVE raw_r
            silu(act_r[:], pm4[0:96, :], wf[0:96, B4C:B4C + 1],
                 None, None, 0, 0, func=AF.Sigmoid)
            NQ = 4
            QW = T // NQ
            for kk in range(CF_ITERS):
                dst = act_r if kk < CF_ITERS - 1 else act_o
                for q in range(NQ):
                    lo, hi = q * QW, (q + 1) * QW
                    pm5, tg5 = new_mm_tile("pm5", width=QW)
                    mm(pm5[0:96, 0:QW],
                       wr[0:96, CDC:CDC + 96],
                       act_r[:, lo:hi], start=True, stop=False)
                    mm(pm5[0:96, 0:QW],
                       wr[0:96, I96C:I96C + 96],
                       raw_r[:, lo:hi], start=False, stop=True)
                    silu(dst[:, lo:hi], pm5[0:96, 0:QW],
                         0.0, dst, tg5, 0, lo, func=AF.Sigmoid)
                    if kk == CF_ITERS - 1:
                        nc.sync.dma_start(out=act_d[:, lo:hi],
                                          in_=act_o[:, lo:hi])

    return nc


def _pack_consts(W1, b1, W2, b2, W3, b3, W4, b4, coupling, decay):
    wf = np.zeros((128, FCOLS), dtype=np.float32)
    for c in range(6):
        wf[:, B1C + c] = b1[c]
    for pr in range(3):
        wf[0:64, B2PC + pr] = b2[2 * pr]
        wf[64:128, B2PC + pr] = b2[2 * pr + 1]
    for c in range(4):
        wf[c * 32:(c + 1) * 32, B3PC] = b3[c]
    for s in range(2):
        wf[64 * s:64 * s + 32, B3P2C] = b3[4]
        wf[64 * s + 32:64 * s + 64, B3P2C] = b3[5]
    wf[0:96, B4C] = np.tile(b4, 16)

    wr = np.zeros((128, RCOLS), dtype=np.float32)
    cd = (decay[:, None] * coupling * CF_K).astype(np.float32)
    for g in range(16):
        wr[6 * g:6 * g + 6, CDC + 6 * g:CDC + 6 * g + 6] = cd
    wr[0:96, I96C:I96C + 96] = np.eye(96, dtype=np.float32)

    wi = np.zeros((128, ICOLS), dtype=np.float32)
    wi[:, IDC:IDC + 128] = np.eye(128, dtype=np.float32)
    for c in range(6):
        wi[0:RES_DIM, W1C + c * 128:W1C + (c + 1) * 128] = W1[c]

    wa = np.zeros((128, ACOLS), dtype=np.float32)
    for c in range(6):
        wa[0:128, W2C + c * 64:W2C + (c + 1) * 64] = W2[c]
    for pr in range(3):
        # odd chamber shifted to out rows 64:127; cols 0:64 stay zero so
        # start=True clears the even chamber's rows for the accumulate
        wa[:, W2BC + pr * 128 + 64:W2BC + (pr + 1) * 128] = W2[2 * pr + 1]
    wa[0:64, W3AC:W3AC + 32] = W3[0]
    wa[64:128, W3AC + 32:W3AC + 64] = W3[1]
    wa[0:64, W3BC + 64:W3BC + 96] = W3[2]
    wa[64:128, W3BC + 96:W3BC + 128] = W3[3]
    wa[0:64, W3CC:W3CC + 32] = W3[4]
    wa[64:128, W3CC + 32:W3CC + 64] = W3[5]

    wz = np.zeros((128, ZCOLS), dtype=np.float32)
    for i in range(16):
        ba = W4AC + 96 * i
        for c in range(4):
            wz[c * 32:(c + 1) * 32, ba + 6 * i + c] = W4[c]
        bb = W4BC + 96 * i
        for s in range(2):
            wz[64 * s:64 * s + 32, bb + 6 * i + 4] = W4[4]
            wz[64 * s + 32:64 * s + 64, bb + 6 * i + 5] = W4[5]
    return (wf, wr, wi.astype(ml_dtypes.bfloat16),
            wa.astype(ml_dtypes.bfloat16), wz.astype(ml_dtypes.bfloat16))


def _unshard(per_core, key):
    """[96, T] group layout -> [BS, 6] per core, concat to [B, 6].

    Chunk 0: sample p*8+n8. Chunks 1-3: 1024 + p*24 + (i-1)*8 + n8.
    Chunks 4-15: 4096 + p*96 + (i-4)*8 + n8."""
    outs = []
    for r in per_core:
        a = r[key].reshape(NCH, 6, 8, 128)             # [i, c, n8, p]
        out = np.empty((BS, 6), dtype=a.dtype)
        out[0:T] = a[0].transpose(2, 1, 0).reshape(T, 6)
        out[T:4 * T] = a[1:4].transpose(3, 0, 2, 1).reshape(3 * T, 6)
        out[4 * T:] = a[4:].transpose(3, 0, 2, 1).reshape(12 * T, 6)
        outs.append(out)
    return np.concatenate(outs, axis=0)


def kernel(res, W1, b1, W2, b2, W3, b3, W4, b4, coupling, decay):
    res = np.asarray(res, dtype=np.float32)
    args = [np.asarray(a, dtype=np.float32)
            for a in (W1, b1, W2, b2, W3, b3, W4, b4, coupling, decay)]
    wf, wr, wi, wa, wz = _pack_consts(*args)

    nc = build_module()
    in_maps = [
        {"res": np.ascontiguousarray(res[i * BS:(i + 1) * BS]),
         "wf": wf, "wr": wr, "wi": wi, "wa": wa, "wz": wz}
        for i in range(NCORES)
    ]
    results = run_bass_kernel_spmd(nc, in_maps, core_ids=list(range(NCORES)))
    act = _unshard(results.results, "act_out")
    raw = _unshard(results.results, "raw_out")
    return act, raw


# revision 21
# speedup vs baseline: 1.2603x; 1.0241x over previous
"""Trainium2 Bass kernel for nn_Chambers (6-tower MLP + coupled sigmoid recurrence).

Data-parallel over 8 NeuronCores: each core processes a 16384-sample shard in
16 chunks of 1024 samples. v2 design:

- bf16 matmul pipeline: res is converted fp32->bf16 on DVE, PE-transposed in
  bf16 (1.0 cycles/row vs 2.0 for fp32), and L1-L4 run as bf16 matmuls with
  chambers packed block-diagonally. All h-activations are bf16 in SBUF.
- L3 chamber pair (4,5) is sample-folded into a [128, 512] tile (samples
  0:511 on partitions 0:63, 512:1023 on 64:127) so its SiLU costs 512 ACT
  rows instead of 1024.
- L4 accumulates raw directly into a persistent [96, 1024] PSUM tile across
  all 16 chunks via per-chunk W4 column stacks (chunk i writes rows 6i:6i+5;
  other rows accumulate zeros). This removes the per-chunk raw bias-copy on
  ACT and all SWDGE assembly DMAs.
- PSUM: 3 rotating [128,1024] matmul tags (6 banks) + the persistent raw
  accumulator (2 banks). Transpose tiles ride the same tag rotation (bf16,
  half a slot); PE "touch" matmuls write into the tag tile being allocated,
  so no scratch bank is needed.
- The 5-step coupled sigmoid recurrence runs on the resident raw tile via a
  block-diagonal [96,96] f32r matmul.

Sync discipline (walrus: at most 1 sem wait + 1 update per instruction):
PSUM tag rotation is pre-observed by 1x2 "touch" matmuls on PE; all other
cross-engine deps resolve to a single auto-added wait because each
instruction's data wait subsumes its WAR wait on the same engine-sem lane.
"""
import numpy as np
import ml_dtypes

import concourse.bass as bass
import concourse.mybir as mybir
from concourse.bass_utils import run_bass_kernel_spmd
from concourse.tile import TileContext
from concourse.tile_scheduler import N_PROCS
from concourse.vector_clock import ScopedClock
from bass_rust import add_dep_helper

F32 = mybir.dt.float32
F32R = mybir.dt.float32r
BF16 = mybir.dt.bfloat16
AF = mybir.ActivationFunctionType
ALU = mybir.AluOpType

B = 131072
NCORES = 8
BS = B // NCORES           # 16384 samples per core
T = 1024                   # chunk (samples)
NCH = BS // T              # 16 chunks
RES_DIM = 100
CF_ITERS = 5
CF_K = 0.02

# wf (fp32) column layout: per-partition bias vectors
B1C = 0        # 6 cols (b1 per chamber, 128 rows)
B2PC = 6       # 3 cols (pair-packed b2: rows 0:64=b2[2pr], 64:128=b2[2pr+1])
B3PC = 9       # 1 col (b3 chambers 0-3 by 32s)
B3P2C = 10     # 1 col (b3 ch4,5 folded twice over 128 rows)
B4C = 11       # 1 col (b4 tiled x16 over 96 rows)
FCOLS = 12

# wr (fp32r): recurrence matrices
CDC = 0        # 96 (block-diag decay*coupling*k)
I96C = 96      # 96 (identity, for raw+delta accumulate)
RCOLS = 192

# wi (bf16): identity + L1 weights (first DMA, gates compute start)
IDC = 0                    # identity [128,128] for PE transpose
W1C = 128                  # 6*128
ICOLS = W1C + 6 * 128

# wa (bf16): L2/L3 weights
W2C = 0                    # 6*64
W2BC = W2C + 6 * 64        # 3*128: odd-chamber W2 shifted to out rows 64:127
W3AC = W2BC + 3 * 128      # 64: pair 0 (both chambers packed on K)
W3BC = W3AC + 64           # 128: pair 1 shifted to out rows 64:127
W3CC = W3BC + 128          # 64: pair 2
ACOLS = W3CC + 64

# wz (bf16): per-chunk W4 stacks
W4AC = 0                   # 16*96 (chambers 0-3)
W4BC = 16 * 96             # 16*96 (ch 4,5; both sample halves)
ZCOLS = 2 * 16 * 96


class TC(TileContext):
    """TileContext with a walrus-compatible epilogue (split final waits)."""

    def _drain_and_barrier(self, tick_clock, wait_clock):
        nc = self.nc
        full = ScopedClock({None: tick_clock.global_clock})
        for scope, vc in full.items():
            for proc in range(N_PROCS):
                t = vc.peek_next(proc) - 1
                if t > 0:
                    sc = ScopedClock()
                    sc.require_at_least(scope, proc, t)
                    w = nc.sync.nop(nofuse=True)
                    wait_clock.add_sem_waits(w.ins, sc)
        for eng in nc.engines.values():
            eng.drain(fusable=False)
        nc.all_engine_barrier(sem_only=True)
        assert self.sems is not None
        popped = nc._tile_sem_poison_stack.pop()
        assert popped is self._sem_poison
        nc.clear_and_free_semaphores(list(self.sems.allocated().values()))
        for eng in nc.engines.values():
            eng.drain(fusable=False)
        nc.all_engine_barrier(sem_only=True)


def _order(after_inst, before_inst):
    if after_inst is not None and before_inst is not None:
        add_dep_helper(after_inst.ins, before_inst.ins, sync=False, reason="order")


def build_module():
    nc = bass.Bass()
    res_d = nc.dram_tensor("res", [BS, RES_DIM], F32, kind="ExternalInput")
    wf_d = nc.dram_tensor("wf", [128, FCOLS], F32, kind="ExternalInput")
    wr_d = nc.dram_tensor("wr", [128, RCOLS], F32R, kind="ExternalInput")
    wi_d = nc.dram_tensor("wi", [128, ICOLS], BF16, kind="ExternalInput")
    wa_d = nc.dram_tensor("wa", [128, ACOLS], BF16, kind="ExternalInput")
    wz_d = nc.dram_tensor("wz", [128, ZCOLS], BF16, kind="ExternalInput")
    raw_d = nc.dram_tensor("raw_out", [96, T], F32, kind="ExternalOutput")
    act_d = nc.dram_tensor("act_out", [96, T], F32, kind="ExternalOutput")

    MMB = 3  # rotating matmul psum tags

    with TC(nc) as tc:
        with (
            tc.tile_pool(name="wconst", bufs=1) as wpool,
            tc.tile_pool(name="sbresb", bufs=1) as sbresb,
            tc.tile_pool(name="sbrt", bufs=1) as sbrt,
            tc.tile_pool(name="sbh", bufs=1) as sbh,
            tc.tile_pool(name="sbrec", bufs=1) as sbrec,
            tc.tile_pool(name="psmm", bufs=1, space="PSUM") as psmm,
            tc.tile_pool(name="psl4", bufs=1, space="PSUM") as psl4,
        ):
            # DMA issue order: transpose identity + W1 + chunk-0 res first so
            # compute starts quickly; L2/L3 weights next; W4 stacks,
            # recurrence weights and later res chunks stream behind.
            wi = wpool.tile([128, ICOLS], BF16)
            nc.sync.dma_start(out=wi[:], in_=wi_d[:])
            res_sb0 = wpool.tile([128, 8 * RES_DIM], F32)
            nc.sync.dma_start(
                out=res_sb0[:],
                in_=res_d[0:T].rearrange("(p n) d -> p (n d)", p=128))
            wf = wpool.tile([128, FCOLS], F32)
            nc.sync.dma_start(out=wf[:], in_=wf_d[:])
            wa = wpool.tile([128, ACOLS], BF16)
            nc.sync.dma_start(out=wa[:], in_=wa_d[:])
            res_sb1 = wpool.tile([128, 3 * 8 * RES_DIM], F32)
            nc.sync.dma_start(
                out=res_sb1[:],
                in_=res_d[T:4 * T].rearrange("(p n) d -> p (n d)", p=128))
            wz = wpool.tile([128, ZCOLS], BF16)
            nc.sync.dma_start(out=wz[:], in_=wz_d[:])
            wr = wpool.tile([128, RCOLS], F32R)
            nc.sync.dma_start(out=wr[:], in_=wr_d[:])
            res_sb = wpool.tile([128, (NCH - 4) * 8 * RES_DIM], F32)
            nc.sync.dma_start(
                out=res_sb[:],
                in_=res_d[4 * T:].rearrange("(p n) d -> p (n d)", p=128))
            ident = wi[:, IDC:IDC + 128]

            raw_sb = sbrec.tile([96, T], F32)
            act_r = sbrec.tile([96, T], F32R)
            act_o = sbrec.tile([96, T], F32)
            raw_r = sbrec.tile([96, T], F32R)
            scr = sbrec.tile([1, 4], F32)
            scrD = sbrec.tile([1, 8], F32)

            pm4 = psl4.tile([96, T], F32)   # persistent raw accumulator

            pe_tail = None
            act_tail = None
            dve_tail = None

            def pe_touch(src_ap, dst_ap):
                """1x2 matmul on PE reading src (absorbing its producer's
                sem) and writing scratch cells at dst (PSUM, f32)."""
                nonlocal pe_tail
                m = nc.tensor.matmul(dst_ap, src_ap[:, 0:1], src_ap[:, 0:2],
                                     start=True, stop=True)
                _order(m, pe_tail)
                pe_tail = m
                return m

            tag_rr = [0]
            tag_state = [None] * MMB

            def new_mm_tile(name, width=T, dtype=F32, parts=128):
                """Allocate the next rotating psum tag tile. Pre-observes the
                tag's previous consumer with a touch matmul that writes into
                the tile itself (safe: the tile's real matmuls re-zero via
                start=True)."""
                tg = tag_rr[0] % MMB
                tag_rr[0] += 1
                st = tag_state[tg]
                if st is not None:
                    tw = psmm.tile([1, 2], F32, tag=f"mm{tg}", bufs=1,
                                   name=f"{name}_tw")
                    tile_, row_, col_ = st
                    pe_touch(tile_[row_:row_ + 1, col_:col_ + 2], tw[0:1, 0:2])
                    tag_state[tg] = None
                t = psmm.tile([parts, width], dtype, tag=f"mm{tg}", bufs=1,
                              name=name)
                return t, tg

            def mm(out_ap, lhs_ap, rhs_ap, **kw):
                nonlocal pe_tail
                m = nc.tensor.matmul(out_ap, lhs_ap, rhs_ap, **kw)
                _order(m, pe_tail)
                pe_tail = m
                return m

            def silu(out_ap, pm_ap, bias_ap, out_tile, tg, row, col,
                     func=AF.Silu):
                nonlocal act_tail
                s = nc.scalar.activation(out_ap, pm_ap, func, bias=bias_ap)
                _order(s, act_tail)
                act_tail = s
                if tg is not None:
                    tag_state[tg] = (out_tile, row, col)
                return s

            def dve(op):
                nonlocal dve_tail
                _order(op, dve_tail)
                dve_tail = op
                return op

            # Startup observes: PE on wi lane, ACT + DVE on wf lane.
            tw0 = psmm.tile([1, 2], F32, tag="mm0", bufs=1, name="warm")
            pe_touch(wi[0:1, 0:2], tw0[0:1, 0:2])
            nc.scalar.activation(scr[0:1, 0:1], wf[0:1, B1C:B1C + 1], AF.Copy)
            nc.vector.tensor_copy(scrD[0:1, 0:1], wf[0:1, B1C:B1C + 1])

            def res_src(i):
                if i == 0:
                    return res_sb0, 0
                if i < 4:
                    return res_sb1, (i - 1) * 8 * RES_DIM
                return res_sb, (i - 4) * 8 * RES_DIM

            def emit_conv(i):
                """DVE: convert chunk i's res slice fp32 -> bf16."""
                rq, coff = res_src(i)
                rb = sbresb.tile([128, 8 * RES_DIM], BF16, tag="rb", bufs=2,
                                 name="rb")
                dve(nc.vector.tensor_copy(
                    rb[:], rq[:, coff:coff + 8 * RES_DIM]))
                return rb

            def emit_tr(i, rb):
                """PE transposes (into a rotating tag slot) + DVE copy ->
                rT [100, 1024] bf16 in SBUF."""
                nonlocal pe_tail
                ptr, tgt = new_mm_tile("ptr", width=T, dtype=BF16,
                                       parts=RES_DIM)
                for nn_ in range(8):
                    t_ = nc.tensor.transpose(
                        ptr[:, nn_ * 128:(nn_ + 1) * 128],
                        rb[:, nn_ * RES_DIM:(nn_ + 1) * RES_DIM],
                        ident,
                    )
                    _order(t_, pe_tail)
                    pe_tail = t_
                rT = sbrt.tile([RES_DIM, T], BF16, tag="rT", bufs=3, name="rT")
                dve(nc.vector.tensor_copy(rT[:], ptr[:]))
                tag_state[tgt] = (rT, 0, 0)
                return rT

            rb_next = emit_conv(0)
            rt_next = emit_tr(0, rb_next)
            pending_l4 = []
            for i in range(NCH):
                rT = rt_next

                # L1: 6 chambers, one [128,1024] psum tile each
                h1s = []
                for cp3 in range(3):
                    ha = sbh.tile([128, T], BF16, tag="h1", bufs=6, name="h1a")
                    hb = sbh.tile([128, T], BF16, tag="h1", bufs=6, name="h1b")
                    pa, ta = new_mm_tile("pm1a")
                    pb, tb = new_mm_tile("pm1b")
                    ca, cb = 2 * cp3, 2 * cp3 + 1
                    for s in range(2):
                        mm(pa[:, s * 512:(s + 1) * 512],
                           wi[0:RES_DIM, W1C + ca * 128:W1C + (ca + 1) * 128],
                           rT[:, s * 512:(s + 1) * 512], start=True, stop=True)
                    for s in range(2):
                        mm(pb[:, s * 512:(s + 1) * 512],
                           wi[0:RES_DIM, W1C + cb * 128:W1C + (cb + 1) * 128],
                           rT[:, s * 512:(s + 1) * 512], start=True, stop=True)
                    silu(ha[:], pa[:], wf[:, B1C + ca:B1C + ca + 1],
                         ha, ta, 0, 0)
                    silu(hb[:], pb[:], wf[:, B1C + cb:B1C + cb + 1],
                         hb, tb, 0, 0)
                    h1s.extend([ha, hb])
                    if cp3 == 0 and pending_l4:
                        # last chunk's L4 rides in PE slack during L1 silus
                        pending_l4.pop(0)()

                # next chunk's res conversion can start as soon as DVE is free
                if i + 1 < NCH:
                    rb_next = emit_conv(i + 1)

                # L2: 3 pairs, both chambers stacked on out partitions
                if i == 0:
                    # one-time observe of the wa DMA lane so the first L2
                    # matmul carries only its h1 data wait; pm4 cells are
                    # safe scratch until L4(0) opens the accumulation group
                    pe_touch(wa[0:1, 0:2], pm4[0:1, 2:4])
                l2t = []
                for pr in range(3):
                    pm2, tg2 = new_mm_tile("pm2")
                    for s in range(2):
                        mm(pm2[:, s * 512:(s + 1) * 512],
                           wa[:, W2BC + pr * 128:W2BC + (pr + 1) * 128],
                           h1s[2 * pr + 1][:, s * 512:(s + 1) * 512],
                           start=True, stop=False)
                        mm(pm2[0:64, s * 512:(s + 1) * 512],
                           wa[:, W2C + 2 * pr * 64:W2C + (2 * pr + 1) * 64],
                           h1s[2 * pr][:, s * 512:(s + 1) * 512],
                           start=False, stop=True)
                    l2t.append((pm2, tg2))
                h2s = []
                for pr in range(3):
                    pm2, tg2 = l2t[pr]
                    h2 = sbh.tile([128, T], BF16, tag="h2", bufs=4, name="h2")
                    silu(h2[:], pm2[:], wf[:, B2PC + pr:B2PC + pr + 1],
                         h2, tg2, 0, 0)
                    h2s.append(h2)

                if i + 1 < NCH:
                    rt_next = emit_tr(i + 1, rb_next)

                # L3: pairs 0,1 merged into one [128,1024] tile; pair 2
                # sample-folded into [128,512]
                h3a = sbh.tile([128, T], BF16, tag="h3a", bufs=3, name="h3a")
                h3b = sbh.tile([128, 512], BF16, tag="h3b", bufs=3, name="h3b")
                # pair-0 (ready first) opens each half's psum group; the
                # full-partition pair-1 matmul lands on pending-zero rows
                pa3, ta3 = new_mm_tile("pm3")
                for s in range(2):
                    mm(pa3[0:64, s * 512:(s + 1) * 512],
                       wa[:, W3AC:W3AC + 64],
                       h2s[0][:, s * 512:(s + 1) * 512], start=True, stop=False)
                for s in range(2):
                    mm(pa3[:, s * 512:(s + 1) * 512],
                       wa[:, W3BC:W3BC + 128],
                       h2s[1][:, s * 512:(s + 1) * 512], start=False, stop=True)
                pc3, tc3 = new_mm_tile("pm3b", width=512)
                mm(pc3[0:64, 0:512], wa[:, W3CC:W3CC + 64],
                   h2s[2][:, 0:512], start=True, stop=False)
                mm(pc3[64:128, 0:512], wa[:, W3CC:W3CC + 64],
                   h2s[2][:, 512:1024], start=False, stop=True)
                silu(h3a[:], pa3[:], wf[:, B3PC:B3PC + 1], h3a, ta3, 0, 0)
                silu(h3b[:], pc3[:], wf[:, B3P2C:B3P2C + 1], h3b, tc3, 0, 0)

                # L4 deferred into the next chunk's L1: accumulate raw
                # rows 6i:6i+6 into the persistent [96,1024] psum tile.
                def emit_l4(i=i, h3a=h3a, h3b=h3b):
                    if i == 0:
                        # observe the W4-stack DMA lane; writing pm4 cells is
                        # safe: the first real matmul start=True re-zeroes
                        pe_touch(wz[0:1, 0:2], pm4[0:1, 0:2])
                    for s in range(2):
                        mm(pm4[0:96, s * 512:(s + 1) * 512],
                           wz[:, W4AC + 96 * i:W4AC + 96 * (i + 1)],
                           h3a[:, s * 512:(s + 1) * 512],
                           start=(i == 0), stop=False)
                        mm(pm4[0:96, s * 512:(s + 1) * 512],
                           wz[64 * s:64 * s + 64,
                              W4BC + 96 * i:W4BC + 96 * (i + 1)],
                           h3b[64 * s:64 * s + 64, 0:512],
                           start=False, stop=(i == NCH - 1))
                pending_l4.append(emit_l4)

            if pending_l4:
                pending_l4.pop(0)()

            # ---- tail: raw materialization + coupled sigmoid recurrence ----
            # Quarter-pipelined: the seed sigmoid (ACT) and the raw+bias
            # path (DVE) stream per 256-col quarter so iteration 1 starts
            # as soon as the first quarter is ready.
            NQ = 4
            QW = T // NQ
            tw1, _ = new_mm_tile("warm2", width=4)
            pe_touch(wr[0:1, 0:2], tw1[0:1, 0:2])      # observe wr DMA lane
            for q in range(NQ):
                lo, hi = q * QW, (q + 1) * QW
                silu(act_r[:, lo:hi], pm4[0:96, lo:hi],
                     wf[0:96, B4C:B4C + 1], None, None, 0, 0, func=AF.Sigmoid)
                dve(nc.vector.tensor_scalar(
                    out=raw_sb[:, lo:hi], in0=pm4[0:96, lo:hi],
                    scalar1=wf[0:96, B4C:B4C + 1], scalar2=None, op0=ALU.add))
                dve(nc.vector.tensor_copy(raw_r[:, lo:hi], raw_sb[:, lo:hi]))
            nc.sync.dma_start(out=raw_d[:], in_=raw_sb[:])
            pe_touch(raw_r[0:1, 0:2], tw1[0:1, 2:4])   # observe DVE raw_r
            for kk in range(CF_ITERS):
                dst = act_r if kk < CF_ITERS - 1 else act_o
                for q in range(NQ):
                    lo, hi = q * QW, (q + 1) * QW
                    pm5, tg5 = new_mm_tile("pm5", width=QW)
                    mm(pm5[0:96, 0:QW],
                       wr[0:96, CDC:CDC + 96],
                       act_r[:, lo:hi], start=True, stop=False)
                    mm(pm5[0:96, 0:QW],
                       wr[0:96, I96C:I96C + 96],
                       raw_r[:, lo:hi], start=False, stop=True)
                    silu(dst[:, lo:hi], pm5[0:96, 0:QW],
                         0.0, dst, tg5, 0, lo, func=AF.Sigmoid)
                    if kk == CF_ITERS - 1:
                        nc.sync.dma_start(out=act_d[:, lo:hi],
                                          in_=act_o[:, lo:hi])

    return nc


def _pack_consts(W1, b1, W2, b2, W3, b3, W4, b4, coupling, decay):
    wf = np.zeros((128, FCOLS), dtype=np.float32)
    for c in range(6):
        wf[:, B1C + c] = b1[c]
    for pr in range(3):
        wf[0:64, B2PC + pr] = b2[2 * pr]
        wf[64:128, B2PC + pr] = b2[2 * pr + 1]
    for c in range(4):
        wf[c * 32:(c + 1) * 32, B3PC] = b3[c]
    for s in range(2):
        wf[64 * s:64 * s + 32, B3P2C] = b3[4]
        wf[64 * s + 32:64 * s + 64, B3P2C] = b3[5]
    wf[0:96, B4C] = np.tile(b4, 16)

    wr = np.zeros((128, RCOLS), dtype=np.float32)
    cd = (decay[:, None] * coupling * CF_K).astype(np.float32)
    for g in range(16):
        wr[6 * g:6 * g + 6, CDC + 6 * g:CDC + 6 * g + 6] = cd
    wr[0:96, I96C:I96C + 96] = np.eye(96, dtype=np.float32)

    wi = np.zeros((128, ICOLS), dtype=np.float32)
    wi[:, IDC:IDC + 128] = np.eye(128, dtype=np.float32)
    for c in range(6):
        wi[0:RES_DIM, W1C + c * 128:W1C + (c + 1) * 128] = W1[c]

    wa = np.zeros((128, ACOLS), dtype=np.float32)
    for c in range(6):
        wa[0:128, W2C + c * 64:W2C + (c + 1) * 64] = W2[c]
    for pr in range(3):
        # odd chamber shifted to out rows 64:127; cols 0:64 stay zero so
        # start=True clears the even chamber's rows for the accumulate
        wa[:, W2BC + pr * 128 + 64:W2BC + (pr + 1) * 128] = W2[2 * pr + 1]
    wa[0:64, W3AC:W3AC + 32] = W3[0]
    wa[64:128, W3AC + 32:W3AC + 64] = W3[1]
    wa[0:64, W3BC + 64:W3BC + 96] = W3[2]
    wa[64:128, W3BC + 96:W3BC + 128] = W3[3]
    wa[0:64, W3CC:W3CC + 32] = W3[4]
    wa[64:128, W3CC + 32:W3CC + 64] = W3[5]

    wz = np.zeros((128, ZCOLS), dtype=np.float32)
    for i in range(16):
        ba = W4AC + 96 * i
        for c in range(4):
            wz[c * 32:(c + 1) * 32, ba + 6 * i + c] = W4[c]
        bb = W4BC + 96 * i
        for s in range(2):
            wz[64 * s:64 * s + 32, bb + 6 * i + 4] = W4[4]
            wz[64 * s + 32:64 * s + 64, bb + 6 * i + 5] = W4[5]
    return (wf, wr, wi.astype(ml_dtypes.bfloat16),
            wa.astype(ml_dtypes.bfloat16), wz.astype(ml_dtypes.bfloat16))


def _unshard(per_core, key):
    """[96, T] group layout -> [BS, 6] per core, concat to [B, 6].

    Chunk 0: sample p*8+n8. Chunks 1-3: 1024 + p*24 + (i-1)*8 + n8.
    Chunks 4-15: 4096 + p*96 + (i-4)*8 + n8."""
    outs = []
    for r in per_core:
        a = r[key].reshape(NCH, 6, 8, 128)             # [i, c, n8, p]
        out = np.empty((BS, 6), dtype=a.dtype)
        out[0:T] = a[0].transpose(2, 1, 0).reshape(T, 6)
        out[T:4 * T] = a[1:4].transpose(3, 0, 2, 1).reshape(3 * T, 6)
        out[4 * T:] = a[4:].transpose(3, 0, 2, 1).reshape(12 * T, 6)
        outs.append(out)
    return np.concatenate(outs, axis=0)


def kernel(res, W1, b1, W2, b2, W3, b3, W4, b4, coupling, decay):
    res = np.asarray(res, dtype=np.float32)
    args = [np.asarray(a, dtype=np.float32)
            for a in (W1, b1, W2, b2, W3, b3, W4, b4, coupling, decay)]
    wf, wr, wi, wa, wz = _pack_consts(*args)

    nc = build_module()
    in_maps = [
        {"res": np.ascontiguousarray(res[i * BS:(i + 1) * BS]),
         "wf": wf, "wr": wr, "wi": wi, "wa": wa, "wz": wz}
        for i in range(NCORES)
    ]
    results = run_bass_kernel_spmd(nc, in_maps, core_ids=list(range(NCORES)))
    act = _unshard(results.results, "act_out")
    raw = _unshard(results.results, "raw_out")
    return act, raw


# revision 23
# speedup vs baseline: 1.2909x; 1.0242x over previous
"""Trainium2 Bass kernel for nn_Chambers (6-tower MLP + coupled sigmoid recurrence).

Data-parallel over 8 NeuronCores: each core processes a 16384-sample shard in
16 chunks of 1024 samples.

- bf16 matmul pipeline: res is converted fp32->bf16 on DVE, PE-transposed in
  bf16 (1.0 cycles/row vs 2.0 for fp32), and L1-L4 run as bf16 matmuls with
  chambers packed block-diagonally. All h-activations are bf16 in SBUF.
- L3 chamber pair (4,5) is sample-folded into a [128, 512] tile so its SiLU
  costs 512 ACT rows instead of 1024.
- L4 accumulates raw directly into a persistent [96, 1024] PSUM tile across
  all 16 chunks via per-chunk W4 column stacks (chunk i writes rows 6i:6i+5;
  other rows accumulate zeros): no per-chunk raw bias-copy on ACT, no
  assembly DMAs.
- PSUM: 3 rotating [128,1024] matmul tags (6 banks) + the persistent raw
  accumulator (2 banks). Transpose tiles ride the tag rotation (bf16, half a
  slot).
- The 5-step coupled sigmoid recurrence runs fully in bf16 on quarter-width
  [96,256] chains with ping-pong act buffers, fed by a block-diagonal [96,96]
  matmul; the final iteration writes fp32.

Sync discipline (walrus allows 1 sem wait per instruction; Tile's dedup
clock advances only on real reads):
- psum tag pre-touches are 1x2 matmuls reading the tag's previous consumer
  and writing into the NEW tile itself (same-tile writes need no sem).
- one tiny ACT touch per chunk reads the previous chunk's h3a, advancing
  ACT's engine clock past every SiLU output-buffer WAW.
- DMA lanes are pre-observed by 1-wait touch reads (PE: wi/wa/wz into pm4
  scratch cells before its group opens; ACT/DVE: wf/res cells into SBUF
  scratch).
"""
import numpy as np
import ml_dtypes

import concourse.bass as bass
import concourse.mybir as mybir
from concourse.bass_utils import run_bass_kernel_spmd
from concourse.tile import TileContext
from concourse.tile_scheduler import N_PROCS
from concourse.vector_clock import ScopedClock
from bass_rust import add_dep_helper

F32 = mybir.dt.float32
BF16 = mybir.dt.bfloat16
AF = mybir.ActivationFunctionType
ALU = mybir.AluOpType

B = 131072
NCORES = 8
BS = B // NCORES           # 16384 samples per core
T = 1024                   # chunk (samples)
NCH = BS // T              # 16 chunks
RES_DIM = 100
CF_ITERS = 5
CF_K = 0.02

# wf (fp32) column layout: per-partition bias vectors
B1C = 0        # 6 cols (b1 per chamber, 128 rows)
B2PC = 6       # 3 cols (pair-packed b2: rows 0:64=b2[2pr], 64:128=b2[2pr+1])
B3PC = 9       # 1 col (b3 chambers 0-3 by 32s)
B3P2C = 10     # 1 col (b3 ch4,5 folded twice over 128 rows)
B4C = 11       # 1 col (b4 tiled x16 over 96 rows)
FCOLS = 12

# wi (bf16): identity + L1 weights (first DMA, gates compute start)
IDC = 0                    # identity [128,128] for PE transpose
W1C = 128                  # 6*128
ICOLS = W1C + 6 * 128

# wa (bf16): L2/L3 weights
W2C = 0                    # 6*64
W2BC = W2C + 6 * 64        # 3*128: odd-chamber W2 shifted to out rows 64:127
W3AC = W2BC + 3 * 128      # 64: pair 0 (both chambers packed on K)
W3BC = W3AC + 64           # 128: pair 1 shifted to out rows 64:127
W3CC = W3BC + 128          # 64: pair 2
ACOLS = W3CC + 64

# wz (bf16): per-chunk W4 stacks + recurrence matrices
W4AC = 0                   # 16*96 (chambers 0-3)
W4BC = 16 * 96             # 16*96 (ch 4,5; both sample halves)
CDC = 2 * 16 * 96          # 96 (block-diag decay*coupling*k)
I96C = CDC + 96            # 96 (identity, for raw+delta accumulate)
ZCOLS = I96C + 96


class TC(TileContext):
    """TileContext with a walrus-compatible epilogue (split final waits)."""

    def _drain_and_barrier(self, tick_clock, wait_clock):
        nc = self.nc
        full = ScopedClock({None: tick_clock.global_clock})
        for scope, vc in full.items():
            for proc in range(N_PROCS):
                t = vc.peek_next(proc) - 1
                if t > 0:
                    sc = ScopedClock()
                    sc.require_at_least(scope, proc, t)
                    w = nc.sync.nop(nofuse=True)
                    wait_clock.add_sem_waits(w.ins, sc)
        for eng in nc.engines.values():
            eng.drain(fusable=False)
        nc.all_engine_barrier(sem_only=True)
        assert self.sems is not None
        popped = nc._tile_sem_poison_stack.pop()
        assert popped is self._sem_poison
        nc.clear_and_free_semaphores(list(self.sems.allocated().values()))
        for eng in nc.engines.values():
            eng.drain(fusable=False)
        nc.all_engine_barrier(sem_only=True)


def _order(after_inst, before_inst):
    if after_inst is not None and before_inst is not None:
        add_dep_helper(after_inst.ins, before_inst.ins, sync=False, reason="order")


def build_module():
    nc = bass.Bass()
    res_d = nc.dram_tensor("res", [BS, RES_DIM], F32, kind="ExternalInput")
    wf_d = nc.dram_tensor("wf", [128, FCOLS], F32, kind="ExternalInput")
    wi_d = nc.dram_tensor("wi", [128, ICOLS], BF16, kind="ExternalInput")
    wa_d = nc.dram_tensor("wa", [128, ACOLS], BF16, kind="ExternalInput")
    wz_d = nc.dram_tensor("wz", [128, ZCOLS], BF16, kind="ExternalInput")
    raw_d = nc.dram_tensor("raw_out", [96, T], F32, kind="ExternalOutput")
    act_d = nc.dram_tensor("act_out", [96, T], F32, kind="ExternalOutput")

    MMB = 3  # rotating matmul psum tags

    with TC(nc) as tc:
        with (
            tc.tile_pool(name="wconst", bufs=1) as wpool,
            tc.tile_pool(name="sbresb", bufs=1) as sbresb,
            tc.tile_pool(name="sbrt", bufs=1) as sbrt,
            tc.tile_pool(name="sbh", bufs=1) as sbh,
            tc.tile_pool(name="sbrec", bufs=1) as sbrec,
            tc.tile_pool(name="psmm", bufs=1, space="PSUM") as psmm,
            tc.tile_pool(name="psl4", bufs=1, space="PSUM") as psl4,
        ):
            # DMA issue order: transpose identity + W1 + chunk-0 res first so
            # compute starts quickly; L2/L3 weights next; W4 stacks and later
            # res chunks stream behind.
            wi = wpool.tile([128, ICOLS], BF16)
            nc.sync.dma_start(out=wi[:], in_=wi_d[:])
            res_sb0 = wpool.tile([128, 8 * RES_DIM], F32)
            nc.sync.dma_start(
                out=res_sb0[:],
                in_=res_d[0:T].rearrange("(p n) d -> p (n d)", p=128))
            wf = wpool.tile([128, FCOLS], F32)
            nc.sync.dma_start(out=wf[:], in_=wf_d[:])
            wa = wpool.tile([128, ACOLS], BF16)
            nc.sync.dma_start(out=wa[:], in_=wa_d[:])
            res_sb1 = wpool.tile([128, 3 * 8 * RES_DIM], F32)
            nc.sync.dma_start(
                out=res_sb1[:],
                in_=res_d[T:4 * T].rearrange("(p n) d -> p (n d)", p=128))
            wz = wpool.tile([128, ZCOLS], BF16)
            nc.sync.dma_start(out=wz[:], in_=wz_d[:])
            res_sb = wpool.tile([128, (NCH - 4) * 8 * RES_DIM], F32)
            nc.sync.dma_start(
                out=res_sb[:],
                in_=res_d[4 * T:].rearrange("(p n) d -> p (n d)", p=128))
            ident = wi[:, IDC:IDC + 128]

            raw_sb = sbrec.tile([96, T], F32)
            raw_r = sbrec.tile([96, T], BF16)
            act_r = sbrec.tile([96, T], BF16)
            act_r2 = sbrec.tile([96, T], BF16)
            act_o = sbrec.tile([96, T], F32)
            scr = sbrec.tile([1, 4], F32)
            scrA = sbrec.tile([1, 64], F32)
            scrD = sbrec.tile([1, 16], F32)

            pm4 = psl4.tile([96, T], F32)   # persistent raw accumulator

            pe_tail = None
            act_tail = None
            dve_tail = None
            acol = [0]
            dcol = [0]

            def pe_touch(src_ap, dst_ap):
                """1x2 matmul on PE reading src (observing its producer's
                sem lane) and writing scratch cells at dst (PSUM, f32)."""
                nonlocal pe_tail
                m = nc.tensor.matmul(dst_ap, src_ap[:, 0:1], src_ap[:, 0:2],
                                     start=True, stop=True)
                _order(m, pe_tail)
                pe_tail = m
                return m

            def act_touch(src_ap):
                """1-elem ACT copy reading src: advances ACT's observed
                clock past src's producer (absorbing later WAW waits)."""
                nonlocal act_tail
                c = acol[0]; acol[0] += 1
                assert c < 64
                s = nc.scalar.activation(scrA[0:1, c:c + 1], src_ap, AF.Copy)
                _order(s, act_tail)
                act_tail = s
                return s

            def dve_touch(src_ap):
                nonlocal dve_tail
                c = dcol[0]; dcol[0] += 1
                assert c < 16
                op = nc.vector.tensor_copy(scrD[0:1, c:c + 1], src_ap)
                _order(op, dve_tail)
                dve_tail = op
                return op

            tag_rr = [0]
            tag_state = [None] * MMB

            def new_mm_tile(name, width=T, dtype=F32, parts=128, touch=True):
                """Allocate the next rotating psum tag tile. For f32 tiles,
                pre-observe the tag's previous consumer with a touch matmul
                writing into the tile itself (same-tile writes carry no WAW
                sem; the real matmuls re-zero via start=True)."""
                tg = tag_rr[0] % MMB
                tag_rr[0] += 1
                t = psmm.tile([parts, width], dtype, tag=f"mm{tg}", bufs=1,
                              name=name)
                st = tag_state[tg]
                tag_state[tg] = None
                if st is not None and touch:
                    assert dtype == F32
                    tile_, row_, col_ = st
                    pe_touch(tile_[row_:row_ + 1, col_:col_ + 2], t[0:1, 0:2])
                return t, tg

            def mm(out_ap, lhs_ap, rhs_ap, **kw):
                nonlocal pe_tail
                m = nc.tensor.matmul(out_ap, lhs_ap, rhs_ap, **kw)
                _order(m, pe_tail)
                pe_tail = m
                return m

            def silu(out_ap, pm_ap, bias_ap, out_tile, tg, row, col,
                     func=AF.Silu):
                nonlocal act_tail
                s = nc.scalar.activation(out_ap, pm_ap, func, bias=bias_ap)
                _order(s, act_tail)
                act_tail = s
                if tg is not None:
                    tag_state[tg] = (out_tile, row, col)
                return s

            def dve(op):
                nonlocal dve_tail
                _order(op, dve_tail)
                dve_tail = op
                return op

            # Startup observes: PE on the wi lane (into a throwaway first
            # tag tile), ACT + DVE on the wf lane.
            tw0, _ = new_mm_tile("warm", width=2, parts=1, touch=False)
            pe_touch(wi[0:1, 0:2], tw0[0:1, 0:2])
            act_touch(wf[0:1, B1C:B1C + 1])
            dve_touch(wf[0:1, B1C:B1C + 1])

            def res_src(i):
                if i == 0:
                    return res_sb0, 0
                if i < 4:
                    return res_sb1, (i - 1) * 8 * RES_DIM
                return res_sb, (i - 4) * 8 * RES_DIM

            def emit_conv(i):
                """DVE: convert chunk i's res slice fp32 -> bf16."""
                if i == 4:
                    # observe the bulk-res DMA lane before conv(4) so it
                    # carries only its buffer WAW
                    dve_touch(res_sb[0:1, 0:1])
                rq, coff = res_src(i)
                rb = sbresb.tile([128, 8 * RES_DIM], BF16, tag="rb", bufs=2,
                                 name="rb")
                dve(nc.vector.tensor_copy(
                    rb[:], rq[:, coff:coff + 8 * RES_DIM]))
                return rb

            def emit_tr(i, rb):
                """PE transposes (into a rotating tag slot) + DVE copy ->
                rT [100, 1024] bf16 in SBUF. No pre-touch: callers order the
                allocation so the tag's WAR is already observed."""
                nonlocal pe_tail
                ptr, tgt = new_mm_tile("ptr", width=T, dtype=BF16,
                                       parts=RES_DIM, touch=False)
                for nn_ in range(8):
                    t_ = nc.tensor.transpose(
                        ptr[:, nn_ * 128:(nn_ + 1) * 128],
                        rb[:, nn_ * RES_DIM:(nn_ + 1) * RES_DIM],
                        ident,
                    )
                    _order(t_, pe_tail)
                    pe_tail = t_
                rT = sbrt.tile([RES_DIM, T], BF16, tag="rT", bufs=3, name="rT")
                dve(nc.vector.tensor_copy(rT[:], ptr[:]))
                tag_state[tgt] = (rT, 0, 0)
                return rT

            rb_next = emit_conv(0)
            rt_next = emit_tr(0, rb_next)
            h3a_prev = [None]
            pending_l4 = []
            for i in range(NCH):
                rT = rt_next

                # one ACT touch absorbs every SiLU output-buffer WAW of this
                # chunk (reads last chunk's h3a; its sem retired during the
                # h3b silu)
                if h3a_prev[0] is not None:
                    act_touch(h3a_prev[0][0:1, 0:1])

                # L1: 6 chambers, one [128,1024] psum tile each
                h1s = []
                for cp3 in range(3):
                    ha = sbh.tile([128, T], BF16, tag="h1", bufs=6, name="h1a")
                    hb = sbh.tile([128, T], BF16, tag="h1", bufs=6, name="h1b")
                    pa, ta = new_mm_tile("pm1a")
                    ca, cb = 2 * cp3, 2 * cp3 + 1
                    for s in range(2):
                        mm(pa[:, s * 512:(s + 1) * 512],
                           wi[0:RES_DIM, W1C + ca * 128:W1C + (ca + 1) * 128],
                           rT[:, s * 512:(s + 1) * 512], start=True, stop=True)
                    pb, tb = new_mm_tile("pm1b")
                    for s in range(2):
                        mm(pb[:, s * 512:(s + 1) * 512],
                           wi[0:RES_DIM, W1C + cb * 128:W1C + (cb + 1) * 128],
                           rT[:, s * 512:(s + 1) * 512], start=True, stop=True)
                    silu(ha[:], pa[:], wf[:, B1C + ca:B1C + ca + 1],
                         ha, ta, 0, 0)
                    silu(hb[:], pb[:], wf[:, B1C + cb:B1C + cb + 1],
                         hb, tb, 0, 0)
                    h1s.extend([ha, hb])
                    if cp3 == 0 and pending_l4:
                        # last chunk's L4 rides in PE slack during L1 silus
                        pending_l4.pop(0)()

                # next chunk's res conversion can start as soon as DVE is free
                if i + 1 < NCH:
                    rb_next = emit_conv(i + 1)

                # L2: 3 pairs, both chambers stacked on out partitions
                if i == 0:
                    # one-time observe of the wa DMA lane (pm4 cells are safe
                    # scratch until L4(0) opens the accumulation group)
                    pe_touch(wa[0:1, 0:2], pm4[0:1, 2:4])
                l2t = []
                for pr in range(3):
                    pm2, tg2 = new_mm_tile("pm2")
                    for s in range(2):
                        mm(pm2[:, s * 512:(s + 1) * 512],
                           wa[:, W2BC + pr * 128:W2BC + (pr + 1) * 128],
                           h1s[2 * pr + 1][:, s * 512:(s + 1) * 512],
                           start=True, stop=False)
                        mm(pm2[0:64, s * 512:(s + 1) * 512],
                           wa[:, W2C + 2 * pr * 64:W2C + (2 * pr + 1) * 64],
                           h1s[2 * pr][:, s * 512:(s + 1) * 512],
                           start=False, stop=True)
                    l2t.append((pm2, tg2))
                h2s = []
                for pr in range(3):
                    pm2, tg2 = l2t[pr]
                    h2 = sbh.tile([128, T], BF16, tag="h2", bufs=4, name="h2")
                    silu(h2[:], pm2[:], wf[:, B2PC + pr:B2PC + pr + 1],
                         h2, tg2, 0, 0)
                    h2s.append(h2)

                # L3: pairs 0,1 merged into one [128,1024] tile; pair-0
                # (ready first) opens each half's psum group and the
                # full-partition pair-1 matmul lands on pending-zero rows
                pa3, ta3 = new_mm_tile("pm3")
                for s in range(2):
                    mm(pa3[0:64, s * 512:(s + 1) * 512],
                       wa[:, W3AC:W3AC + 64],
                       h2s[0][:, s * 512:(s + 1) * 512], start=True, stop=False)
                for s in range(2):
                    mm(pa3[:, s * 512:(s + 1) * 512],
                       wa[:, W3BC:W3BC + 128],
                       h2s[1][:, s * 512:(s + 1) * 512], start=False, stop=True)

                # transposes for the next chunk slot in here: their tag WAR
                # (silu pm2_1) was just observed by pa3's matmuls
                if i + 1 < NCH:
                    rt_next = emit_tr(i + 1, rb_next)

                # pair 2 sample-folded into [128,512]
                h3a = sbh.tile([128, T], BF16, tag="h3a", bufs=3, name="h3a")
                h3b = sbh.tile([128, 512], BF16, tag="h3b", bufs=3, name="h3b")
                pc3, tc3 = new_mm_tile("pm3b", width=512)
                mm(pc3[0:64, 0:512], wa[:, W3CC:W3CC + 64],
                   h2s[2][:, 0:512], start=True, stop=False)
                mm(pc3[64:128, 0:512], wa[:, W3CC:W3CC + 64],
                   h2s[2][:, 512:1024], start=False, stop=True)
                silu(h3a[:], pa3[:], wf[:, B3PC:B3PC + 1], h3a, ta3, 0, 0)
                silu(h3b[:], pc3[:], wf[:, B3P2C:B3P2C + 1], h3b, tc3, 0, 0)
                h3a_prev[0] = h3a

                # L4 deferred into the next chunk's L1: accumulate raw rows
                # 6i:6i+6 into the persistent [96,1024] psum tile
                def emit_l4(i=i, h3a=h3a, h3b=h3b):
                    if i == 0:
                        # observe the W4-stack DMA lane; pm4 cells are safe:
                        # the first real matmul start=True re-zeroes
                        pe_touch(wz[0:1, 0:2], pm4[0:1, 0:2])
                    for s in range(2):
                        mm(pm4[0:96, s * 512:(s + 1) * 512],
                           wz[:, W4AC + 96 * i:W4AC + 96 * (i + 1)],
                           h3a[:, s * 512:(s + 1) * 512],
                           start=(i == 0), stop=False)
                        mm(pm4[0:96, s * 512:(s + 1) * 512],
                           wz[64 * s:64 * s + 64,
                              W4BC + 96 * i:W4BC + 96 * (i + 1)],
                           h3b[64 * s:64 * s + 64, 0:512],
                           start=False, stop=(i == NCH - 1))
                pending_l4.append(emit_l4)

            if pending_l4:
                pending_l4.pop(0)()

            # ---- tail: raw materialization + coupled sigmoid recurrence ----
            # Quarter-pipelined: the raw+bias path (DVE) and seed sigmoid
            # (ACT) stream per 256-col quarter so iteration 1 starts as soon
            # as the first quarter is ready. Recurrence state is bf16 with
            # ping-pong act buffers; the final iteration writes fp32.
            NQ = 4
            QW = T // NQ
            for q in range(NQ):
                lo, hi = q * QW, (q + 1) * QW
                dve(nc.vector.tensor_scalar(
                    out=raw_sb[:, lo:hi], in0=pm4[0:96, lo:hi],
                    scalar1=wf[0:96, B4C:B4C + 1], scalar2=None, op0=ALU.add))
                dve(nc.vector.tensor_copy(raw_r[:, lo:hi], raw_sb[:, lo:hi]))
                silu(act_r[:, lo:hi], pm4[0:96, lo:hi],
                     wf[0:96, B4C:B4C + 1], None, None, 0, 0, func=AF.Sigmoid)
            nc.sync.dma_start(out=raw_d[:], in_=raw_sb[:])

            bufs = [act_r, act_r2]
            for kk in range(CF_ITERS):
                src = bufs[kk % 2]
                dst = bufs[(kk + 1) % 2] if kk < CF_ITERS - 1 else act_o
                if kk >= 1:
                    # advance ACT's clock past the previous iteration's
                    # sigmoids (absorbs the ping-pong WAW two iters back)
                    act_touch(src[0:1, 0:1])
                for q in range(NQ):
                    lo, hi = q * QW, (q + 1) * QW
                    pm5, tg5 = new_mm_tile("pm5", width=QW)
                    mm(pm5[0:96, 0:QW],
                       wz[0:96, CDC:CDC + 96],
                       src[:, lo:hi], start=True, stop=False)
                    mm(pm5[0:96, 0:QW],
                       wz[0:96, I96C:I96C + 96],
                       raw_r[:, lo:hi], start=False, stop=True)
                    silu(dst[:, lo:hi], pm5[0:96, 0:QW],
                         0.0, dst, tg5, 0, lo, func=AF.Sigmoid)
                    if kk == CF_ITERS - 1:
                        nc.sync.dma_start(out=act_d[:, lo:hi],
                                          in_=act_o[:, lo:hi])

    return nc


def _pack_consts(W1, b1, W2, b2, W3, b3, W4, b4, coupling, decay):
    wf = np.zeros((128, FCOLS), dtype=np.float32)
    for c in range(6):
        wf[:, B1C + c] = b1[c]
    for pr in range(3):
        wf[0:64, B2PC + pr] = b2[2 * pr]
        wf[64:128, B2PC + pr] = b2[2 * pr + 1]
    for c in range(4):
        wf[c * 32:(c + 1) * 32, B3PC] = b3[c]
    for s in range(2):
        wf[64 * s:64 * s + 32, B3P2C] = b3[4]
        wf[64 * s + 32:64 * s + 64, B3P2C] = b3[5]
    wf[0:96, B4C] = np.tile(b4, 16)

    wi = np.zeros((128, ICOLS), dtype=np.float32)
    wi[:, IDC:IDC + 128] = np.eye(128, dtype=np.float32)
    for c in range(6):
        wi[0:RES_DIM, W1C + c * 128:W1C + (c + 1) * 128] = W1[c]

    wa = np.zeros((128, ACOLS), dtype=np.float32)
    for c in range(6):
        wa[0:128, W2C + c * 64:W2C + (c + 1) * 64] = W2[c]
    for pr in range(3):
        # odd chamber shifted to out rows 64:127; cols 0:64 stay zero so
        # start=True clears the even chamber's rows for the accumulate
        wa[:, W2BC + pr * 128 + 64:W2BC + (pr + 1) * 128] = W2[2 * pr + 1]
    wa[0:64, W3AC:W3AC + 32] = W3[0]
    wa[64:128, W3AC + 32:W3AC + 64] = W3[1]
    wa[0:64, W3BC + 64:W3BC + 96] = W3[2]
    wa[64:128, W3BC + 96:W3BC + 128] = W3[3]
    wa[0:64, W3CC:W3CC + 32] = W3[4]
    wa[64:128, W3CC + 32:W3CC + 64] = W3[5]

    wz = np.zeros((128, ZCOLS), dtype=np.float32)
    for i in range(16):
        ba = W4AC + 96 * i
        for c in range(4):
            wz[c * 32:(c + 1) * 32, ba + 6 * i + c] = W4[c]
        bb = W4BC + 96 * i
        for s in range(2):
            wz[64 * s:64 * s + 32, bb + 6 * i + 4] = W4[4]
            wz[64 * s + 32:64 * s + 64, bb + 6 * i + 5] = W4[5]
    cd = (decay[:, None] * coupling * CF_K).astype(np.float32)
    for g in range(16):
        wz[6 * g:6 * g + 6, CDC + 6 * g:CDC + 6 * g + 6] = cd
    wz[0:96, I96C:I96C + 96] = np.eye(96, dtype=np.float32)
    return (wf, wi.astype(ml_dtypes.bfloat16), wa.astype(ml_dtypes.bfloat16),
            wz.astype(ml_dtypes.bfloat16))


def _unshard(per_core, key):
    """[96, T] group layout -> [BS, 6] per core, concat to [B, 6].

    Chunk 0: sample p*8+n8. Chunks 1-3: 1024 + p*24 + (i-1)*8 + n8.
    Chunks 4-15: 4096 + p*96 + (i-4)*8 + n8."""
    outs = []
    for r in per_core:
        a = r[key].reshape(NCH, 6, 8, 128)             # [i, c, n8, p]
        out = np.empty((BS, 6), dtype=a.dtype)
        out[0:T] = a[0].transpose(2, 1, 0).reshape(T, 6)
        out[T:4 * T] = a[1:4].transpose(3, 0, 2, 1).reshape(3 * T, 6)
        out[4 * T:] = a[4:].transpose(3, 0, 2, 1).reshape(12 * T, 6)
        outs.append(out)
    return np.concatenate(outs, axis=0)


def kernel(res, W1, b1, W2, b2, W3, b3, W4, b4, coupling, decay):
    res = np.asarray(res, dtype=np.float32)
    args = [np.asarray(a, dtype=np.float32)
            for a in (W1, b1, W2, b2, W3, b3, W4, b4, coupling, decay)]
    wf, wi, wa, wz = _pack_consts(*args)

    nc = build_module()
    in_maps = [
        {"res": np.ascontiguousarray(res[i * BS:(i + 1) * BS]),
         "wf": wf, "wi": wi, "wa": wa, "wz": wz}
        for i in range(NCORES)
    ]
    results = run_bass_kernel_spmd(nc, in_maps, core_ids=list(range(NCORES)))
    act = _unshard(results.results, "act_out")
    raw = _unshard(results.results, "raw_out")
    return act, raw
